# revision 1
# baseline (speedup 1.0000x reference)
"""Trainium2 Bass kernel for a 6-layer decoder LM (D=768, H=12, S=2048, B=2, V=32000).

Sharding (8 NeuronCores, one uniform SPMD program):
  core c: batch b=c//4, row-block r=c%4 (rows [r*512,(r+1)*512) of its batch),
  heads {3r,3r+1,3r+2} for attention, vocab shard [c*4000,(c+1)*4000) for the
  output projection.
  Per layer: LN1 on own rows -> PE-transpose -> AllGather h^T within the batch
  group -> QKV for the core's 3 heads over the full sequence -> causal
  attention in transposed-score layout (keys on partitions, exp on ScalarE,
  softmax denominator via an appended ones-column on V, normalization via a
  rank-1 PE broadcast) -> Wo partial sums -> ReduceScatter -> residual;
  LN2 -> MLP on own rows -> residual.  Final: LNf -> AllGather h_f^T over all
  8 cores -> vocab-parallel output matmul -> [4096, 4000] f32 per core.

All matmuls run in bf16 (weights are cast host-side); residual stream, LN
statistics and PSUM accumulation stay f32.
"""
import math
from contextlib import ExitStack

import numpy as np
import ml_dtypes

import concourse.bacc as bacc
import concourse.bass as bass
import concourse.mybir as mybir
import concourse.tile as tile
from concourse.bass_utils import run_bass_kernel_spmd

V, D, H, L, EPS = 32000, 768, 12, 6, 1e-5
DH = D // H          # 64
FF = 4 * D           # 3072
B, S = 2, 2048
NC = 8
RPC = 512            # rows per core
HPC = 3              # heads per core
VPC = V // NC        # 4000
VCH = 500            # vocab chunk per psum
P = 128
SQRT_D = math.sqrt(D)
F32 = mybir.dt.float32
BF16 = mybir.dt.bfloat16
I32 = mybir.dt.int32
AF = mybir.ActivationFunctionType
ALU = mybir.AluOpType
AX = mybir.AxisListType

GROUPS4 = [[0, 1, 2, 3], [4, 5, 6, 7]]
GROUPS8 = [list(range(NC))]


def _bf(x):
    return np.ascontiguousarray(np.asarray(x, np.float32)).astype(ml_dtypes.bfloat16)


def _build_masks():
    # column block m (of 4): mask[p, m*512 + j] = 1 if j >= m*128 + p else 0
    m = np.zeros((P, 4 * 512), np.float32)
    j = np.arange(512)[None, :]
    p = np.arange(P)[:, None]
    for mm in range(4):
        m[:, mm * 512:(mm + 1) * 512] = (j >= mm * P + p)
    return m.astype(ml_dtypes.bfloat16)


def _pos_encoding():
    pos = np.arange(S, dtype=np.float32)[:, None]
    div = np.exp(np.arange(0, D, 2, dtype=np.float32) * (-math.log(10000.0) / D))
    pe = np.zeros((S, D), dtype=np.float32)
    pe[:, 0::2] = np.sin(pos * div)
    pe[:, 1::2] = np.cos(pos * div)
    return pe


class Flags:
    def __init__(self, qkv_bias, b1_bias, b2_bias, out_bias, ln1_aff, ln2_aff, lnf_aff):
        self.qkv_bias = qkv_bias
        self.b1_bias = b1_bias
        self.b2_bias = b2_bias
        self.out_bias = out_bias
        self.ln1_aff = ln1_aff
        self.ln2_aff = ln2_aff
        self.lnf_aff = lnf_aff

    def key(self):
        return (self.qkv_bias, self.b1_bias, self.b2_bias, self.out_bias,
                self.ln1_aff, self.ln2_aff, self.lnf_aff)


def build_nc(flags: Flags, taps=()):
    nc = bacc.Bacc("TRN2", target_bir_lowering=False, debug=False, num_devices=NC)

    emb_in = nc.dram_tensor("emb", [V, D], F32, kind="ExternalInput")
    tok_in = nc.dram_tensor("tok", [4, P, 1], I32, kind="ExternalInput")
    pe_in = nc.dram_tensor("pe", [RPC, D], F32, kind="ExternalInput")
    wqkv_in = nc.dram_tensor("wqkv", [L, D, D], BF16, kind="ExternalInput")
    wo_in = nc.dram_tensor("wo", [L, HPC * DH, D], BF16, kind="ExternalInput")
    w1_in = nc.dram_tensor("w1", [L, D, FF], BF16, kind="ExternalInput")
    w2_in = nc.dram_tensor("w2", [L, FF, D], BF16, kind="ExternalInput")
    wout_in = nc.dram_tensor("wout", [D, VPC], BF16, kind="ExternalInput")
    amask_in = nc.dram_tensor("amask", [P, 4 * 512], BF16, kind="ExternalInput")
    ident_in = nc.dram_tensor("ident", [P, P], BF16, kind="ExternalInput")
    if flags.qkv_bias:
        qkvb_in = nc.dram_tensor("qkvb", [L, P, 6], F32, kind="ExternalInput")
    if flags.b1_bias:
        b1_in = nc.dram_tensor("b1b", [L, P, 24], F32, kind="ExternalInput")
    if flags.b2_bias or flags.out_bias:
        ones_in = nc.dram_tensor("onesrow", [1, P], BF16, kind="ExternalInput")
    if flags.b2_bias:
        b2_in = nc.dram_tensor("b2b", [L, 1, D], BF16, kind="ExternalInput")
    if flags.out_bias:
        bout_in = nc.dram_tensor("boutb", [1, VPC], BF16, kind="ExternalInput")
    if flags.ln1_aff:
        ln1_in = nc.dram_tensor("ln1wb", [L, 2, P, D], F32, kind="ExternalInput")
    if flags.ln2_aff:
        ln2_in = nc.dram_tensor("ln2wb", [L, 2, P, D], F32, kind="ExternalInput")
    if flags.lnf_aff:
        lnf_in = nc.dram_tensor("lnfwb", [2, P, D], F32, kind="ExternalInput")

    logits_out = nc.dram_tensor("logits", [NC * RPC, VPC], F32, kind="ExternalOutput")
    tap_outs = {}
    for t in taps:
        if t.startswith("x"):
            tap_outs[t] = nc.dram_tensor(f"tap_{t}", [RPC, D], F32, kind="ExternalOutput")
        elif t == "qkvT":
            tap_outs[t] = nc.dram_tensor("tap_qkvT", [D, S], F32, kind="ExternalOutput")
        elif t == "oT":
            tap_outs[t] = nc.dram_tensor("tap_oT", [HPC * DH, S], F32, kind="ExternalOutput")
        elif t == "hT":
            tap_outs[t] = nc.dram_tensor("tap_hT", [D, S], F32, kind="ExternalOutput")

    with tile.TileContext(nc) as tc:
        with ExitStack() as ctx:
            const = ctx.enter_context(tc.tile_pool(name="const", bufs=1))
            dram = ctx.enter_context(tc.tile_pool(name="dram", bufs=1, space="DRAM"))
            xres = ctx.enter_context(tc.tile_pool(name="xres", bufs=1))
            sbw = ctx.enter_context(tc.tile_pool(name="sbw", bufs=1))    # weights
            sba = ctx.enter_context(tc.tile_pool(name="sba", bufs=1))    # activations
            sbe = ctx.enter_context(tc.tile_pool(name="sbe", bufs=2))    # evict/stage
            ps = ctx.enter_context(tc.tile_pool(name="ps", bufs=8, space="PSUM"))

            def pst(shape=(P, 512), dtype=F32):
                return ps.tile(list(shape), dtype, tag="ps", name="pst")

            # ---------------- constants ----------------
            ident = const.tile([P, P], BF16, name="ident")
            nc.sync.dma_start(ident[:], ident_in[:])
            amask = const.tile([P, 4 * 512], BF16, name="amask")
            nc.sync.dma_start(amask[:], amask_in[:])
            ones64 = const.tile([1, DH], BF16, name="ones64")
            nc.vector.memset(ones64[:], 1.0)
            if flags.b2_bias or flags.out_bias:
                onesrow = const.tile([1, P], BF16, name="onesrow")
                nc.sync.dma_start(onesrow[:], ones_in[:])

            def ln_stats(xt, out_bf, wb_tiles=None):
                """LayerNorm of x tile [128, D] f32 -> out_bf [128, D] bf16."""
                mu = sbe.tile([P, 1], F32, tag="mu", bufs=2)
                nc.vector.reduce_sum(mu[:], xt[:], axis=AX.X)
                nc.vector.tensor_scalar_mul(mu[:], mu[:], 1.0 / D)
                xm = sbe.tile([P, D], F32, tag="xm", bufs=1)
                nc.vector.tensor_scalar_sub(xm[:], xt[:], mu[:])
                sq = sbe.tile([P, D], F32, tag="hn", bufs=1, name="sq")
                nc.vector.tensor_tensor(out=sq[:], in0=xm[:], in1=xm[:], op=ALU.mult)
                var = sbe.tile([P, 1], F32, tag="var", bufs=2)
                nc.vector.reduce_sum(var[:], sq[:], axis=AX.X)
                nc.vector.tensor_scalar(var[:], var[:], 1.0 / D, EPS,
                                        op0=ALU.mult, op1=ALU.add)
                rec = sbe.tile([P, 1], F32, tag="rec", bufs=2)
                nc.vector.reciprocal(rec[:], var[:])
                rstd = sbe.tile([P, 1], F32, tag="rstd", bufs=2)
                nc.scalar.sqrt(rstd[:], rec[:])
                if wb_tiles is None:
                    nc.vector.tensor_scalar_mul(out_bf[:], xm[:], rstd[:])
                else:
                    wt, bt = wb_tiles
                    hn = sbe.tile([P, D], F32, tag="hn", bufs=1)
                    nc.vector.tensor_scalar_mul(hn[:], xm[:], rstd[:])
                    nc.vector.tensor_tensor(out=hn[:], in0=hn[:], in1=wt[:], op=ALU.mult)
                    nc.vector.tensor_tensor(out=out_bf[:], in0=hn[:], in1=bt[:], op=ALU.add)

            def transpose_to(dst_sb, src_sb, rows=P):
                """dst_sb [cols, rows] <- src_sb [rows, cols].T  (cols<=128)."""
                cols = src_sb.shape[-1]
                pt = pst((P, P), BF16)
                nc.tensor.transpose(pt[:cols, :rows], src_sb, ident[:rows, :rows])
                nc.scalar.copy(dst_sb, pt[:cols, :rows])

            def ln_transpose_block(wb_tiles, hT_tiles):
                """LN x -> h bf16, then fill hT_tiles[dt][:, st*128...] = h^T."""
                for st in range(4):
                    h_bf = sba.tile([P, D], BF16, tag="h_bf", bufs=2)
                    ln_stats(x_tiles[st], h_bf, wb_tiles)
                    for dt in range(6):
                        transpose_to(hT_tiles[dt][:, st * P:(st + 1) * P],
                                     h_bf[:, dt * P:(dt + 1) * P])

            # ---------------- embedding + positional encoding ----------------
            x_tiles = []
            for st in range(4):
                xt = xres.tile([P, D], F32, name=f"x{st}", tag=f"x{st}", bufs=1)
                x_tiles.append(xt)
                idx = sbe.tile([P, 1], I32, tag="idx", bufs=2)
                nc.sync.dma_start(idx[:], tok_in[st])
                gt = sba.tile([P, D], F32, tag="gt", bufs=1)
                nc.gpsimd.indirect_dma_start(
                    out=gt[:], out_offset=None, in_=emb_in[:],
                    in_offset=bass.IndirectOffsetOnAxis(ap=idx[:, :1], axis=0),
                )
                pt_ = sba.tile([P, D], F32, tag="pt_", bufs=1)
                nc.sync.dma_start(pt_[:], pe_in[st * P:(st + 1) * P, :])
                nc.vector.tensor_scalar_mul(xt[:], gt[:], SQRT_D)
                nc.vector.tensor_tensor(out=xt[:], in0=xt[:], in1=pt_[:], op=ALU.add)

            if "x0" in tap_outs:
                for st in range(4):
                    nc.sync.dma_start(tap_outs["x0"][st * P:(st + 1) * P, :], x_tiles[st][:])

            hT_loc = dram.tile([D, RPC], BF16, name="hT_loc")
            hT_gath = dram.tile([4 * D, RPC], BF16, name="hT_gath")
            xpart = dram.tile([S, D], F32, name="xpart")
            xdelta = dram.tile([RPC, D], F32, name="xdelta")
            hfT_loc = dram.tile([D, RPC], BF16, name="hfT_loc")
            hfT_gath = dram.tile([NC * D, RPC], BF16, name="hfT_gath")

            # ---------------- layers ----------------
            for l in range(L):
                # ---- LN1 + transpose + AllGather ----
                if flags.ln1_aff:
                    lw = sba.tile([P, D], F32, tag="lnw", bufs=2)
                    lb = sba.tile([P, D], F32, tag="lnb", bufs=2)
                    nc.sync.dma_start(lw[:], ln1_in[l, 0])
                    nc.sync.dma_start(lb[:], ln1_in[l, 1])
                hTl = [sba.tile([P, RPC], BF16, tag=f"hTl{dt}", bufs=1, name=f"hTl{dt}") for dt in range(6)]
                ln_transpose_block((lw, lb) if flags.ln1_aff else None, hTl)
                for dt in range(6):
                    nc.sync.dma_start(hT_loc[dt * P:(dt + 1) * P, :], hTl[dt][:])
                nc.gpsimd.collective_compute(
                    "AllGather", ALU.bypass, replica_groups=GROUPS4,
                    ins=[hT_loc[:].opt()], outs=[hT_gath[:].opt()],
                )
                # ---- QKV projection (own heads, full sequence) ----
                wq_sb = []
                for kt in range(6):
                    t = sbw.tile([P, D], BF16, tag=f"wq{kt}", bufs=1)
                    wq_sb.append(t)
                    nc.sync.dma_start(t[:], wqkv_in[l, kt * P:(kt + 1) * P, :])
                if flags.qkv_bias:
                    qb = sba.tile([P, 6], F32, tag="qb", bufs=2)
                    nc.sync.dma_start(qb[:], qkvb_in[l])
                qkvT = [sba.tile([P, S], BF16, tag=f"qkvT{mt}", bufs=1, name=f"qkvT{mt}")
                        for mt in range(6)]
                for sc in range(4):
                    hTc = []
                    for kt in range(6):
                        t = sba.tile([P, 512], BF16, tag=f"hTc{kt}", bufs=2)
                        hTc.append(t)
                        nc.sync.dma_start(
                            t[:], hT_gath[sc * D + kt * P: sc * D + (kt + 1) * P, :])
                    for mt in range(6):
                        pq = pst()
                        for kt in range(6):
                            nc.tensor.matmul(
                                pq[:], wq_sb[kt][:, mt * P:(mt + 1) * P], hTc[kt][:],
                                start=(kt == 0), stop=(kt == 5),
                            )
                        dst = qkvT[mt][:, sc * 512:(sc + 1) * 512]
                        if flags.qkv_bias:
                            nc.scalar.activation(dst, pq[:], AF.Identity,
                                                 bias=qb[:, mt:mt + 1])
                        else:
                            nc.scalar.copy(dst, pq[:])

                if "qkvT" in tap_outs and l == 0:
                    for mt in range(6):
                        tf = sbe.tile([P, S], F32, tag="tapf", bufs=1)
                        nc.vector.tensor_copy(tf[:], qkvT[mt][:])
                        nc.sync.dma_start(tap_outs["qkvT"][mt * P:(mt + 1) * P, :], tf[:])

                def hrow(base, h):
                    off = base + h * DH
                    return off // P, off % P

                # ---- attention ----
                oT0 = sba.tile([P, S], BF16, tag="oT0", bufs=1)
                oT1 = sba.tile([DH, S], BF16, tag="oT1", bufs=1)
                for h in range(HPC):
                    qt_i, qr = hrow(0, h)
                    kt_i, kr = hrow(256, h)
                    vt_i, vr = hrow(512, h)
                    vaug = []
                    for kb in range(16):
                        va = sba.tile([P, DH + 1], BF16, tag=f"vaug{kb}", bufs=1)
                        vaug.append(va)
                        pt = pst((P, P), BF16)
                        nc.tensor.transpose(
                            pt[:P, :DH],
                            qkvT[vt_i][vr:vr + DH, kb * P:(kb + 1) * P],
                            ident[vr:vr + DH, vr:vr + DH])
                        nc.scalar.copy(va[:, :DH], pt[:P, :DH])
                        nc.vector.memset(va[:, DH:DH + 1], 1.0)
                    for qc in range(4):
                        po = pst((DH + 1, 512))
                        nkt = 4 * qc + 4
                        for kb in range(nkt):
                            psc = pst()
                            nc.tensor.matmul(
                                psc[:],
                                qkvT[kt_i][kr:kr + DH, kb * P:(kb + 1) * P],
                                qkvT[qt_i][qr:qr + DH, qc * 512:(qc + 1) * 512],
                                start=True, stop=True,
                            )
                            ex = sba.tile([P, 512], BF16, tag="ex", bufs=3)
                            nc.scalar.activation(ex[:], psc[:], AF.Exp, scale=0.125)
                            m = kb - 4 * qc
                            if m >= 0:
                                nc.vector.tensor_tensor(
                                    out=ex[:], in0=ex[:],
                                    in1=amask[:, m * 512:(m + 1) * 512], op=ALU.mult)
                            nc.tensor.matmul(
                                po[:], vaug[kb][:], ex[:],
                                start=(kb == 0), stop=(kb == nkt - 1),
                            )
                        rec = sbe.tile([1, 512], F32, tag="recd", bufs=2)
                        nc.vector.reciprocal(rec[:], po[DH:DH + 1, :])
                        recb = sbe.tile([1, 512], BF16, tag="recb", bufs=2)
                        nc.vector.tensor_copy(recb[:], rec[:])
                        pb = pst((DH, 512))
                        nc.tensor.matmul(pb[:], ones64[:], recb[:], start=True, stop=True)
                        bc = sbe.tile([DH, 512], F32, tag="bc", bufs=1)
                        nc.scalar.copy(bc[:], pb[:])
                        dst = oT0[h * DH:(h + 1) * DH, qc * 512:(qc + 1) * 512] \
                            if h < 2 else oT1[:, qc * 512:(qc + 1) * 512]
                        nc.vector.tensor_tensor(out=dst, in0=po[:DH, :], in1=bc[:],
                                                op=ALU.mult)

                if "oT" in tap_outs and l == 0:
                    tf = sbe.tile([P, S], F32, tag="tapf", bufs=1)
                    nc.vector.tensor_copy(tf[:], oT0[:])
                    nc.sync.dma_start(tap_outs["oT"][0:P, :], tf[:])
                    tf2 = sbe.tile([P, S], F32, tag="tapf", bufs=1)
                    nc.vector.tensor_copy(tf2[:DH, :], oT1[:])
                    nc.sync.dma_start(tap_outs["oT"][P:P + DH, :], tf2[:DH, :])

                # ---- Wo partial + ReduceScatter ----
                wo0 = sbw.tile([P, D], BF16, tag="wo0", bufs=2)
                wo1 = sbw.tile([DH, D], BF16, tag="wo1", bufs=2)
                nc.sync.dma_start(wo0[:], wo_in[l, 0:P, :])
                nc.sync.dma_start(wo1[:], wo_in[l, P:P + DH, :])
                for st in range(16):
                    sl = slice(st * P, (st + 1) * P)
                    pxa = pst()
                    pxb = pst((P, 256))
                    for (px, c0, cn) in ((pxa, 0, 512), (pxb, 512, 256)):
                        nc.tensor.matmul(px[:, :cn], oT0[:, sl], wo0[:, c0:c0 + cn],
                                         start=True, stop=False)
                        nc.tensor.matmul(px[:, :cn], oT1[:, sl], wo1[:, c0:c0 + cn],
                                         start=False, stop=True)
                    ev = sbe.tile([P, D], F32, tag="xev", bufs=2)
                    nc.scalar.copy(ev[:, 0:512], pxa[:, :])
                    nc.scalar.copy(ev[:, 512:768], pxb[:, :])
                    nc.sync.dma_start(xpart[sl, :], ev[:])
                nc.gpsimd.collective_compute(
                    "ReduceScatter", ALU.add, replica_groups=GROUPS4,
                    ins=[xpart[:].opt()], outs=[xdelta[:].opt()],
                )
                for st in range(4):
                    xd = sba.tile([P, D], F32, tag="xd", bufs=2)
                    nc.sync.dma_start(xd[:], xdelta[st * P:(st + 1) * P, :])
                    nc.vector.tensor_tensor(out=x_tiles[st][:], in0=x_tiles[st][:],
                                            in1=xd[:], op=ALU.add)
                if f"x{l}a" in tap_outs:
                    for st in range(4):
                        nc.sync.dma_start(tap_outs[f"x{l}a"][st * P:(st + 1) * P, :],
                                          x_tiles[st][:])

                # ---- LN2 + transpose ----
                if flags.ln2_aff:
                    lw2 = sba.tile([P, D], F32, tag="lnw", bufs=2)
                    lb2 = sba.tile([P, D], F32, tag="lnb", bufs=2)
                    nc.sync.dma_start(lw2[:], ln2_in[l, 0])
                    nc.sync.dma_start(lb2[:], ln2_in[l, 1])
                h2T = [sba.tile([P, RPC], BF16, tag=f"h2T{dt}", bufs=1, name=f"h2T{dt}") for dt in range(6)]
                ln_transpose_block((lw2, lb2) if flags.ln2_aff else None, h2T)

                # ---- MLP: h1T = relu(W1^T @ h2 (+b1)); x += h1 @ W2 (+b2) ----
                h1T_all = {}
                if flags.b1_bias:
                    b1t = sba.tile([P, 24], F32, tag="b1t", bufs=2)
                    nc.sync.dma_start(b1t[:], b1_in[l])
                if flags.b2_bias:
                    b2t = sba.tile([1, D], BF16, tag="b2t", bufs=2)
                    nc.sync.dma_start(b2t[:], b2_in[l])
                pxs = []
                for st in range(4):
                    pxa = pst()
                    pxb = pst((P, 256))
                    pxs.append((pxa, pxb))
                h1T_all = {}
                for sg in range(2):
                    for ffh in range(2):
                        if sg == 0:
                            w1_sb = []
                            for kt in range(6):
                                t = sbw.tile([P, FF // 2], BF16, tag=f"w1{kt}",
                                             bufs=1, name=f"w1_{kt}")
                                w1_sb.append(t)
                                nc.sync.dma_start(
                                    t[:], w1_in[l, kt * P:(kt + 1) * P,
                                                ffh * (FF // 2):(ffh + 1) * (FF // 2)])
                            for ft in range(12):
                                fft = ffh * 12 + ft
                                ph = pst()
                                for kt in range(6):
                                    nc.tensor.matmul(
                                        ph[:], w1_sb[kt][:, ft * P:(ft + 1) * P],
                                        h2T[kt][:],
                                        start=(kt == 0), stop=(kt == 5))
                                t = sba.tile([P, RPC], BF16, tag=f"h1T{ft}",
                                             bufs=2, name=f"h1T{fft}")
                                if flags.b1_bias:
                                    nc.scalar.activation(t[:], ph[:], AF.Relu,
                                                         bias=b1t[:, fft:fft + 1])
                                else:
                                    nc.scalar.activation(t[:], ph[:], AF.Relu)
                                h1T_all[fft] = t
                        for ft in range(12):
                            fft = ffh * 12 + ft
                            w2t = sbw.tile([P, D], BF16, tag="w2s", bufs=3)
                            nc.sync.dma_start(w2t[:], w2_in[l, fft * P:(fft + 1) * P, :])
                            for st in (2 * sg, 2 * sg + 1):
                                pxa, pxb = pxs[st]
                                for (px, c0, cn) in ((pxa, 0, 512), (pxb, 512, 256)):
                                    nc.tensor.matmul(
                                        px[:, :cn],
                                        h1T_all[fft][:, st * P:(st + 1) * P],
                                        w2t[:, c0:c0 + cn],
                                        start=(ffh == 0 and ft == 0),
                                        stop=(ffh == 1 and ft == 11
                                              and not flags.b2_bias))
                if flags.b2_bias:
                    for st in range(4):
                        pxa, pxb = pxs[st]
                        for (px, c0, cn) in ((pxa, 0, 512), (pxb, 512, 256)):
                            nc.tensor.matmul(px[:, :cn], onesrow[:], b2t[:, c0:c0 + cn],
                                             start=False, stop=True)
                for st in range(4):
                    pxa, pxb = pxs[st]
                    nc.vector.tensor_tensor(out=x_tiles[st][:, 0:512],
                                            in0=x_tiles[st][:, 0:512],
                                            in1=pxa[:, :], op=ALU.add)
                    nc.vector.tensor_tensor(out=x_tiles[st][:, 512:768],
                                            in0=x_tiles[st][:, 512:768],
                                            in1=pxb[:, :], op=ALU.add)
                if f"x{l}m" in tap_outs:
                    for st in range(4):
                        nc.sync.dma_start(tap_outs[f"x{l}m"][st * P:(st + 1) * P, :],
                                          x_tiles[st][:])

            # ---------------- final LN + AllGather + output projection ----------------
            if flags.lnf_aff:
                lwf = sba.tile([P, D], F32, tag="lnw", bufs=2)
                lbf = sba.tile([P, D], F32, tag="lnb", bufs=2)
                nc.sync.dma_start(lwf[:], lnf_in[0])
                nc.sync.dma_start(lbf[:], lnf_in[1])
            hfTl = [sba.tile([P, RPC], BF16, tag=f"hTl{dt}", bufs=1, name=f"hfTl{dt}") for dt in range(6)]
            ln_transpose_block((lwf, lbf) if flags.lnf_aff else None, hfTl)
            for dt in range(6):
                nc.sync.dma_start(hfT_loc[dt * P:(dt + 1) * P, :], hfTl[dt][:])
            nc.gpsimd.collective_compute(
                "AllGather", ALU.bypass, replica_groups=GROUPS8,
                ins=[hfT_loc[:].opt()], outs=[hfT_gath[:].opt()],
            )

            if flags.out_bias:
                boutt = sba.tile([1, VPC], BF16, tag="boutt", bufs=1)
                nc.sync.dma_start(boutt[:], bout_in[:])
            for sc in range(NC):
                hfc = []
                for kt in range(6):
                    t = sba.tile([P, RPC], BF16, tag=f"hfc{kt}", bufs=1)
                    hfc.append(t)
                    nc.sync.dma_start(
                        t[:], hfT_gath[sc * D + kt * P: sc * D + (kt + 1) * P, :])
                for vc in range(NC):
                    wv_sb = []
                    for kt in range(6):
                        t = sbw.tile([P, VCH], BF16, tag=f"wv{kt}", bufs=2)
                        wv_sb.append(t)
                        nc.sync.dma_start(t[:], wout_in[kt * P:(kt + 1) * P,
                                                        vc * VCH:(vc + 1) * VCH])
                    for st in range(4):
                        pl = pst((P, VCH))
                        for kt in range(6):
                            nc.tensor.matmul(
                                pl[:], hfc[kt][:, st * P:(st + 1) * P], wv_sb[kt][:],
                                start=(kt == 0),
                                stop=(kt == 5 and not flags.out_bias))
                        if flags.out_bias:
                            nc.tensor.matmul(pl[:], onesrow[:],
                                             boutt[:, vc * VCH:(vc + 1) * VCH],
                                             start=False, stop=True)
                        lv = sbe.tile([P, VCH], F32, tag="lv", bufs=2)
                        if st % 2 == 0:
                            nc.scalar.copy(lv[:], pl[:])
                        else:
                            nc.vector.tensor_copy(lv[:], pl[:])
                        nc.sync.dma_start(
                            logits_out[(sc * 4 + st) * P:(sc * 4 + st + 1) * P,
                                       vc * VCH:(vc + 1) * VCH], lv[:])

    nc.compile()
    return nc


_NC_CACHE = {}


def _prep_inputs(inputs, flags):
    tokens = np.asarray(inputs["tokens"]).astype(np.int32)
    emb = np.asarray(inputs["emb"], np.float32)
    Wq, Wk, Wv = (np.asarray(inputs[k], np.float32) for k in ("Wq", "Wk", "Wv"))
    Wo = np.asarray(inputs["Wo"], np.float32)
    bq, bk, bv = (np.asarray(inputs[k], np.float32) for k in ("bq", "bk", "bv"))
    W1, W2 = np.asarray(inputs["W1"], np.float32), np.asarray(inputs["W2"], np.float32)
    b1, b2 = np.asarray(inputs["b1"], np.float32), np.asarray(inputs["b2"], np.float32)
    Wout = np.asarray(inputs["Wout"], np.float32)
    bout = np.asarray(inputs["bout"], np.float32)

    pe_full = _pos_encoding()
    amask = _build_masks()
    ident = np.eye(P, dtype=np.float32).astype(ml_dtypes.bfloat16)
    w1_bf = _bf(W1)
    w2_bf = _bf(W2)

    def col_tiles(vec, n_tiles):
        # [n] -> [128, n_tiles] padded (per-partition bias layout)
        npad = n_tiles * P
        v = np.zeros(npad, np.float32)
        v[:vec.shape[0]] = vec
        return np.ascontiguousarray(v.reshape(n_tiles, P).T)

    in_maps = []
    for c in range(NC):
        b, r = c // 4, c % 4
        rows = slice(r * RPC, (r + 1) * RPC)
        heads = slice(3 * r * DH, 3 * (r + 1) * DH)
        vsh = slice(c * VPC, (c + 1) * VPC)
        wqkv = np.zeros((L, D, D), np.float32)
        wqkv[:, :, 0:192] = Wq[:, :, heads]
        wqkv[:, :, 256:448] = Wk[:, :, heads]
        wqkv[:, :, 512:704] = Wv[:, :, heads]
        m = {
            "emb": emb,
            "tok": np.ascontiguousarray(tokens[b, rows].reshape(4, P, 1)),
            "pe": np.ascontiguousarray(pe_full[rows]),
            "wqkv": _bf(wqkv),
            "wo": _bf(Wo[:, heads, :]),
            "w1": w1_bf,
            "w2": w2_bf,
            "wout": _bf(Wout[:, vsh]),
            "amask": amask,
            "ident": ident,
        }
        if flags.qkv_bias:
            qkvb = np.zeros((L, D), np.float32)
            qkvb[:, 0:192] = bq[:, heads]
            qkvb[:, 256:448] = bk[:, heads]
            qkvb[:, 512:704] = bv[:, heads]
            m["qkvb"] = np.stack([col_tiles(qkvb[l], 6) for l in range(L)])
        if flags.b1_bias:
            m["b1b"] = np.stack([col_tiles(b1[l], 24) for l in range(L)])
        if flags.b2_bias or flags.out_bias:
            m["onesrow"] = np.ones((1, P), ml_dtypes.bfloat16)
        if flags.b2_bias:
            m["b2b"] = _bf(b2.reshape(L, 1, D))
        if flags.out_bias:
            m["boutb"] = _bf(bout[vsh].reshape(1, VPC))
        if flags.ln1_aff:
            m["ln1wb"] = np.ascontiguousarray(np.stack([
                np.broadcast_to(np.asarray(inputs["ln1_w"], np.float32)[:, None, :], (L, P, D)),
                np.broadcast_to(np.asarray(inputs["ln1_b"], np.float32)[:, None, :], (L, P, D))],
                axis=1))
        if flags.ln2_aff:
            m["ln2wb"] = np.ascontiguousarray(np.stack([
                np.broadcast_to(np.asarray(inputs["ln2_w"], np.float32)[:, None, :], (L, P, D)),
                np.broadcast_to(np.asarray(inputs["ln2_b"], np.float32)[:, None, :], (L, P, D))],
                axis=1))
        if flags.lnf_aff:
            m["lnfwb"] = np.ascontiguousarray(np.stack([
                np.broadcast_to(np.asarray(inputs["lnf_w"], np.float32)[None, :], (P, D)),
                np.broadcast_to(np.asarray(inputs["lnf_b"], np.float32)[None, :], (P, D))],
                axis=0))
        in_maps.append(m)
    return in_maps


def make_flags(inputs):
    ln1_w = np.asarray(inputs["ln1_w"]); ln1_b = np.asarray(inputs["ln1_b"])
    ln2_w = np.asarray(inputs["ln2_w"]); ln2_b = np.asarray(inputs["ln2_b"])
    lnf_w = np.asarray(inputs["lnf_w"]); lnf_b = np.asarray(inputs["lnf_b"])
    return Flags(
        qkv_bias=bool(np.any(np.asarray(inputs["bq"])) or np.any(np.asarray(inputs["bk"]))
                      or np.any(np.asarray(inputs["bv"]))),
        b1_bias=bool(np.any(np.asarray(inputs["b1"]))),
        b2_bias=bool(np.any(np.asarray(inputs["b2"]))),
        out_bias=bool(np.any(np.asarray(inputs["bout"]))),
        ln1_aff=not (np.all(ln1_w == 1) and not np.any(ln1_b)),
        ln2_aff=not (np.all(ln2_w == 1) and not np.any(ln2_b)),
        lnf_aff=not (np.all(lnf_w == 1) and not np.any(lnf_b)),
    )


def run(inputs, taps=()):
    """Run and return (logits, results) — taps add debug outputs."""
    flags = make_flags(inputs)
    key = (flags.key(), tuple(sorted(taps)))
    if key not in _NC_CACHE:
        _NC_CACHE[key] = build_nc(flags, taps)
    nc = _NC_CACHE[key]
    in_maps = _prep_inputs(inputs, flags)
    res = run_bass_kernel_spmd(nc, in_maps, core_ids=list(range(NC)))
    parts = [res.results[c]["logits"].reshape(B, S, VPC) for c in range(NC)]
    return np.concatenate(parts, axis=-1), res


def kernel(**inputs) -> np.ndarray:
    logits, _ = run(inputs)
    return logits



# revision 10
# speedup vs baseline: 1.0840x; 1.0840x over previous
"""Trainium2 Bass kernel for a 6-layer decoder LM (D=768, H=12, S=2048, B=2, V=32000).

Sharding (8 NeuronCores, one uniform SPMD program):
  core c: batch b=c//4, r=c%4. The core owns the four STRIDED 128-row blocks
  {r, 4+r, 8+r, 12+r} (of 16 blocks per batch) for the residual stream / LN /
  MLP, heads {3r,3r+1,3r+2} for attention, vocab shard [c*4000,(c+1)*4000)
  for the output projection.
  Per layer: LN1 on own rows -> PE-transpose -> AllGather h^T within the batch
  group -> QKV for the core's 3 heads over the full sequence (evicted into
  global column order via a strided AP so attention sees tokens in sequence
  order) -> causal attention in transposed-score layout (keys on partitions,
  exp on ScalarE, softmax denominator via an appended ones-column on V,
  normalization via a rank-1 PE broadcast) -> Wo partial sums in bf16 ->
  4x chunked bf16 ReduceScatter (strided ownership makes each 512-row chunk
  scatter to exactly the owner's local block; chunking keeps each collective
  on the fast <1MB path) -> residual; LN2 -> MLP on own rows -> residual.
  Final: LNf -> AllGather h_f^T over all 8 cores -> vocab-parallel output
  matmul -> [4096, 4000] f32 per core.

All matmuls run in bf16 (weights are cast host-side); residual stream, LN
statistics and PSUM accumulation stay f32.
"""
import math
from contextlib import ExitStack

import numpy as np
import ml_dtypes

import concourse.bacc as bacc
import concourse.bass as bass
import concourse.mybir as mybir
import concourse.tile as tile
from concourse.bass_utils import run_bass_kernel_spmd

V, D, H, L, EPS = 32000, 768, 12, 6, 1e-5
DH = D // H          # 64
FF = 4 * D           # 3072
B, S = 2, 2048
NC = 8
RPC = 512            # rows per core
HPC = 3              # heads per core
VPC = V // NC        # 4000
VCH = 500            # vocab chunk per psum
P = 128
SQRT_D = math.sqrt(D)
F32 = mybir.dt.float32
BF16 = mybir.dt.bfloat16
I32 = mybir.dt.int32
AF = mybir.ActivationFunctionType
ALU = mybir.AluOpType
AX = mybir.AxisListType

GROUPS4 = [[0, 1, 2, 3], [4, 5, 6, 7]]
GROUPS8 = [list(range(NC))]


def _bf(x):
    return np.ascontiguousarray(np.asarray(x, np.float32)).astype(ml_dtypes.bfloat16)


def _build_masks():
    # column block m (of 4): mask[p, m*512 + j] = 1 if j >= m*128 + p else 0
    m = np.zeros((P, 4 * 512), np.float32)
    j = np.arange(512)[None, :]
    p = np.arange(P)[:, None]
    for mm in range(4):
        m[:, mm * 512:(mm + 1) * 512] = (j >= mm * P + p)
    return m.astype(ml_dtypes.bfloat16)


def _pos_encoding():
    pos = np.arange(S, dtype=np.float32)[:, None]
    div = np.exp(np.arange(0, D, 2, dtype=np.float32) * (-math.log(10000.0) / D))
    pe = np.zeros((S, D), dtype=np.float32)
    pe[:, 0::2] = np.sin(pos * div)
    pe[:, 1::2] = np.cos(pos * div)
    return pe


class Flags:
    def __init__(self, qkv_bias, b1_bias, b2_bias, out_bias, ln1_aff, ln2_aff, lnf_aff):
        self.qkv_bias = qkv_bias
        self.b1_bias = b1_bias
        self.b2_bias = b2_bias
        self.out_bias = out_bias
        self.ln1_aff = ln1_aff
        self.ln2_aff = ln2_aff
        self.lnf_aff = lnf_aff

    def key(self):
        return (self.qkv_bias, self.b1_bias, self.b2_bias, self.out_bias,
                self.ln1_aff, self.ln2_aff, self.lnf_aff)


def build_nc(flags: Flags, taps=()):
    nc = bacc.Bacc("TRN2", target_bir_lowering=False, debug=False, num_devices=NC)

    emb_in = nc.dram_tensor("emb", [V, D], F32, kind="ExternalInput")
    tok_in = nc.dram_tensor("tok", [4, P, 1], I32, kind="ExternalInput")
    pe_in = nc.dram_tensor("pe", [RPC, D], F32, kind="ExternalInput")
    wqkv_in = nc.dram_tensor("wqkv", [L, D, D], BF16, kind="ExternalInput")
    wo_in = nc.dram_tensor("wo", [L, HPC * DH, D], BF16, kind="ExternalInput")
    w1_in = nc.dram_tensor("w1", [L, D, FF], BF16, kind="ExternalInput")
    w2_in = nc.dram_tensor("w2", [L, FF, D], BF16, kind="ExternalInput")
    wout_in = nc.dram_tensor("wout", [D, VPC], BF16, kind="ExternalInput")
    amask_in = nc.dram_tensor("amask", [P, 4 * 512], BF16, kind="ExternalInput")
    ident_in = nc.dram_tensor("ident", [P, P], BF16, kind="ExternalInput")
    if flags.qkv_bias:
        qkvb_in = nc.dram_tensor("qkvb", [L, P, 6], F32, kind="ExternalInput")
    if flags.b1_bias:
        b1_in = nc.dram_tensor("b1b", [L, P, 24], F32, kind="ExternalInput")
    if flags.b2_bias or flags.out_bias:
        ones_in = nc.dram_tensor("onesrow", [1, P], BF16, kind="ExternalInput")
    if flags.b2_bias:
        b2_in = nc.dram_tensor("b2b", [L, 1, D], BF16, kind="ExternalInput")
    if flags.out_bias:
        bout_in = nc.dram_tensor("boutb", [1, VPC], BF16, kind="ExternalInput")
    if flags.ln1_aff:
        ln1_in = nc.dram_tensor("ln1wb", [L, 2, P, D], F32, kind="ExternalInput")
    if flags.ln2_aff:
        ln2_in = nc.dram_tensor("ln2wb", [L, 2, P, D], F32, kind="ExternalInput")
    if flags.lnf_aff:
        lnf_in = nc.dram_tensor("lnfwb", [2, P, D], F32, kind="ExternalInput")

    logits_out = nc.dram_tensor("logits_v2", [NC * RPC, VPC], F32, kind="ExternalOutput")
    tap_outs = {}
    for t in taps:
        if t.startswith("x"):
            tap_outs[t] = nc.dram_tensor(f"tap_{t}", [RPC, D], F32, kind="ExternalOutput")
        elif t == "qkvT":
            tap_outs[t] = nc.dram_tensor("tap_qkvT", [D, S], F32, kind="ExternalOutput")
        elif t == "oT":
            tap_outs[t] = nc.dram_tensor("tap_oT", [HPC * DH, S], F32, kind="ExternalOutput")
        elif t == "hT":
            tap_outs[t] = nc.dram_tensor("tap_hT", [D, S], F32, kind="ExternalOutput")

    with tile.TileContext(nc) as tc:
        with ExitStack() as ctx:
            const = ctx.enter_context(tc.tile_pool(name="const", bufs=1))
            dram = ctx.enter_context(tc.tile_pool(name="dram", bufs=1, space="DRAM"))
            xres = ctx.enter_context(tc.tile_pool(name="xres", bufs=1))
            sbw = ctx.enter_context(tc.tile_pool(name="sbw", bufs=1))    # weights
            sba = ctx.enter_context(tc.tile_pool(name="sba", bufs=1))    # activations
            sbe = ctx.enter_context(tc.tile_pool(name="sbe", bufs=2))    # evict/stage
            ps = ctx.enter_context(tc.tile_pool(name="ps", bufs=8, space="PSUM"))

            def pst(shape=(P, 512), dtype=F32):
                return ps.tile(list(shape), dtype, tag="ps", name="pst")

            # ---------------- constants ----------------
            ident = const.tile([P, P], BF16, name="ident")
            nc.sync.dma_start(ident[:], ident_in[:])
            amask = const.tile([P, 4 * 512], BF16, name="amask")
            nc.sync.dma_start(amask[:], amask_in[:])
            ones64 = const.tile([1, DH], BF16, name="ones64")
            nc.vector.memset(ones64[:], 1.0)
            if flags.b2_bias or flags.out_bias:
                onesrow = const.tile([1, P], BF16, name="onesrow")
                nc.sync.dma_start(onesrow[:], ones_in[:])

            def ln_stats(xt, out_bf, wb_tiles=None):
                """LayerNorm of x tile [128, D] f32 -> out_bf [128, D] bf16."""
                mu = sbe.tile([P, 1], F32, tag="mu", bufs=2)
                nc.vector.reduce_sum(mu[:], xt[:], axis=AX.X)
                nc.vector.tensor_scalar_mul(mu[:], mu[:], 1.0 / D)
                xm = sbe.tile([P, D], F32, tag="xm", bufs=1)
                nc.vector.tensor_scalar_sub(xm[:], xt[:], mu[:])
                sq = sbe.tile([P, D], F32, tag="hn", bufs=1, name="sq")
                nc.vector.tensor_tensor(out=sq[:], in0=xm[:], in1=xm[:], op=ALU.mult)
                var = sbe.tile([P, 1], F32, tag="var", bufs=2)
                nc.vector.reduce_sum(var[:], sq[:], axis=AX.X)
                nc.vector.tensor_scalar(var[:], var[:], 1.0 / D, EPS,
                                        op0=ALU.mult, op1=ALU.add)
                rec = sbe.tile([P, 1], F32, tag="rec", bufs=2)
                nc.vector.reciprocal(rec[:], var[:])
                rstd = sbe.tile([P, 1], F32, tag="rstd", bufs=2)
                nc.scalar.sqrt(rstd[:], rec[:])
                if wb_tiles is None:
                    nc.vector.tensor_scalar_mul(out_bf[:], xm[:], rstd[:])
                else:
                    wt, bt = wb_tiles
                    hn = sbe.tile([P, D], F32, tag="hn", bufs=1)
                    nc.vector.tensor_scalar_mul(hn[:], xm[:], rstd[:])
                    nc.vector.tensor_tensor(out=hn[:], in0=hn[:], in1=wt[:], op=ALU.mult)
                    nc.vector.tensor_tensor(out=out_bf[:], in0=hn[:], in1=bt[:], op=ALU.add)

            def transpose_to(dst_sb, src_sb, rows=P):
                """dst_sb [cols, rows] <- src_sb [rows, cols].T  (cols<=128)."""
                cols = src_sb.shape[-1]
                pt = pst((P, P), BF16)
                nc.tensor.transpose(pt[:cols, :rows], src_sb, ident[:rows, :rows])
                nc.scalar.copy(dst_sb, pt[:cols, :rows])

            def ln_transpose_block(wb_tiles, hT_tiles):
                """LN x -> h bf16, then fill hT_tiles[dt][:, st*128...] = h^T."""
                for st in range(4):
                    h_bf = sba.tile([P, D], BF16, tag="h_bf", bufs=2)
                    ln_stats(x_tiles[st], h_bf, wb_tiles)
                    for dt in range(6):
                        transpose_to(hT_tiles[dt][:, st * P:(st + 1) * P],
                                     h_bf[:, dt * P:(dt + 1) * P])

            # ---------------- embedding + positional encoding ----------------
            x_tiles = []
            for st in range(4):
                xt = xres.tile([P, D], F32, name=f"x{st}", tag=f"x{st}", bufs=1)
                x_tiles.append(xt)
                idx = sbe.tile([P, 1], I32, tag="idx", bufs=2)
                nc.sync.dma_start(idx[:], tok_in[st])
                gt = sba.tile([P, D], F32, tag="gt", bufs=1)
                nc.gpsimd.indirect_dma_start(
                    out=gt[:], out_offset=None, in_=emb_in[:],
                    in_offset=bass.IndirectOffsetOnAxis(ap=idx[:, :1], axis=0),
                )
                pt_ = sba.tile([P, D], F32, tag="pt_", bufs=1)
                nc.sync.dma_start(pt_[:], pe_in[st * P:(st + 1) * P, :])
                nc.vector.tensor_scalar_mul(xt[:], gt[:], SQRT_D)
                nc.vector.tensor_tensor(out=xt[:], in0=xt[:], in1=pt_[:], op=ALU.add)

            if "x0" in tap_outs:
                for st in range(4):
                    nc.sync.dma_start(tap_outs["x0"][st * P:(st + 1) * P, :], x_tiles[st][:])

            hT_loc = dram.tile([D, RPC], BF16, name="hT_loc")
            hT_gath = dram.tile([4 * D, RPC], BF16, name="hT_gath")
            xpart = dram.tile([S, D], BF16, name="xpart")
            xdelta = dram.tile([RPC, D], BF16, name="xdelta")
            hfT_loc = dram.tile([D, RPC], BF16, name="hfT_loc")
            hfT_gath = dram.tile([NC * D, RPC], BF16, name="hfT_gath")

            # ---------------- layers ----------------
            for l in range(L):
                # ---- LN1 + transpose + AllGather ----
                if flags.ln1_aff:
                    lw = sba.tile([P, D], F32, tag="lnw", bufs=2)
                    lb = sba.tile([P, D], F32, tag="lnb", bufs=2)
                    nc.sync.dma_start(lw[:], ln1_in[l, 0])
                    nc.sync.dma_start(lb[:], ln1_in[l, 1])
                hTl = [sba.tile([P, RPC], BF16, tag=f"hTl{dt}", bufs=1, name=f"hTl{dt}") for dt in range(6)]
                ln_transpose_block((lw, lb) if flags.ln1_aff else None, hTl)
                for dt in range(6):
                    nc.sync.dma_start(hT_loc[dt * P:(dt + 1) * P, :], hTl[dt][:])
                nc.gpsimd.collective_compute(
                    "AllGather", ALU.bypass, replica_groups=GROUPS4,
                    ins=[hT_loc[:].opt()], outs=[hT_gath[:].opt()],
                )
                # ---- QKV projection (own heads, full sequence) ----
                wq_sb = []
                for kt in range(6):
                    t = sbw.tile([P, D], BF16, tag=f"wq{kt}", bufs=1)
                    wq_sb.append(t)
                    nc.sync.dma_start(t[:], wqkv_in[l, kt * P:(kt + 1) * P, :])
                if flags.qkv_bias:
                    qb = sba.tile([P, 6], F32, tag="qb", bufs=2)
                    nc.sync.dma_start(qb[:], qkvb_in[l])
                # qkvT layout [P, c, s, i]: column (c*4+s)*128+i is global token
                # block 4c+s, so attention sees tokens in global sequence order
                # even though each source core's rows are the strided blocks
                # {s, 4+s, 8+s, 12+s}.
                qkvT = [sba.tile([P, 4, 4, P], BF16, tag=f"qkvT{mt}", bufs=1,
                                 name=f"qkvT{mt}")
                        for mt in range(6)]
                for sc in range(4):
                    hTc = []
                    for kt in range(6):
                        t = sba.tile([P, 512], BF16, tag=f"hTc{kt}", bufs=2)
                        hTc.append(t)
                        nc.sync.dma_start(
                            t[:], hT_gath[sc * D + kt * P: sc * D + (kt + 1) * P, :])
                    for mt in range(6):
                        pq = pst()
                        for kt in range(6):
                            nc.tensor.matmul(
                                pq[:], wq_sb[kt][:, mt * P:(mt + 1) * P], hTc[kt][:],
                                start=(kt == 0), stop=(kt == 5),
                            )
                        dst = qkvT[mt][:, :, sc, :]
                        if flags.qkv_bias:
                            nc.scalar.activation(dst, pq[:], AF.Identity,
                                                 bias=qb[:, mt:mt + 1])
                        else:
                            nc.scalar.copy(dst, pq[:])

                def hrow(base, h):
                    off = base + h * DH
                    return off // P, off % P

                # ---- attention ----
                oT0 = sba.tile([P, S], BF16, tag="oT0", bufs=1)
                oT1 = sba.tile([DH, S], BF16, tag="oT1", bufs=1)
                for h in range(HPC):
                    qt_i, qr = hrow(0, h)
                    kt_i, kr = hrow(256, h)
                    vt_i, vr = hrow(512, h)
                    vaug = []
                    for kb in range(16):
                        va = sba.tile([P, DH + 1], BF16, tag=f"vaug{kb}", bufs=1)
                        vaug.append(va)
                        pt = pst((P, P), BF16)
                        nc.tensor.transpose(
                            pt[:P, :DH],
                            qkvT[vt_i][vr:vr + DH, kb // 4, kb % 4, :],
                            ident[vr:vr + DH, vr:vr + DH])
                        nc.scalar.copy(va[:, :DH], pt[:P, :DH])
                        nc.vector.memset(va[:, DH:DH + 1], 1.0)
                    for qc in range(4):
                        po = pst((DH + 1, 512))
                        nkt = 4 * qc + 4
                        for kb in range(nkt):
                            psc = pst()
                            nc.tensor.matmul(
                                psc[:],
                                qkvT[kt_i][kr:kr + DH, kb // 4, kb % 4, :],
                                qkvT[qt_i][qr:qr + DH, qc, :, :],
                                start=True, stop=True,
                            )
                            ex = sba.tile([P, 512], BF16, tag="ex", bufs=3)
                            nc.scalar.activation(ex[:], psc[:], AF.Exp, scale=0.125)
                            m = kb - 4 * qc
                            if m >= 0:
                                nc.vector.tensor_tensor(
                                    out=ex[:], in0=ex[:],
                                    in1=amask[:, m * 512:(m + 1) * 512], op=ALU.mult)
                            nc.tensor.matmul(
                                po[:], vaug[kb][:], ex[:],
                                start=(kb == 0), stop=(kb == nkt - 1),
                            )
                        rec = sbe.tile([1, 512], F32, tag="recd", bufs=2)
                        nc.vector.reciprocal(rec[:], po[DH:DH + 1, :])
                        recb = sbe.tile([1, 512], BF16, tag="recb", bufs=2)
                        nc.vector.tensor_copy(recb[:], rec[:])
                        pb = pst((DH, 512))
                        nc.tensor.matmul(pb[:], ones64[:], recb[:], start=True, stop=True)
                        bc = sbe.tile([DH, 512], F32, tag="bc", bufs=1)
                        nc.scalar.copy(bc[:], pb[:])
                        dst = oT0[h * DH:(h + 1) * DH, qc * 512:(qc + 1) * 512] \
                            if h < 2 else oT1[:, qc * 512:(qc + 1) * 512]
                        nc.vector.tensor_tensor(out=dst, in0=po[:DH, :], in1=bc[:],
                                                op=ALU.mult)

                # ---- Wo partial (bf16) + 4x chunked bf16 ReduceScatter ----
                wo0 = sbw.tile([P, D], BF16, tag="wo0", bufs=2)
                wo1 = sbw.tile([DH, D], BF16, tag="wo1", bufs=2)
                nc.sync.dma_start(wo0[:], wo_in[l, 0:P, :])
                nc.sync.dma_start(wo1[:], wo_in[l, P:P + DH, :])
                for st in range(16):
                    sl = slice(st * P, (st + 1) * P)
                    pxa = pst()
                    pxb = pst((P, 256))
                    for (px, c0, cn) in ((pxa, 0, 512), (pxb, 512, 256)):
                        nc.tensor.matmul(px[:, :cn], oT0[:, sl], wo0[:, c0:c0 + cn],
                                         start=True, stop=False)
                        nc.tensor.matmul(px[:, :cn], oT1[:, sl], wo1[:, c0:c0 + cn],
                                         start=False, stop=True)
                    ev = sbe.tile([P, D], BF16, tag="xev", bufs=2)
                    nc.scalar.copy(ev[:, 0:512], pxa[:, :])
                    nc.vector.tensor_copy(ev[:, 512:768], pxb[:, :])
                    nc.sync.dma_start(xpart[sl, :], ev[:])
                # Each 512-row chunk c holds global blocks 4c..4c+3; the RS
                # scatters 128-row quarters so core r receives global block
                # 4c+r == its local block c. Chunking also keeps each
                # collective input below the ~1MB slow-path cliff.
                for cch in range(4):
                    nc.gpsimd.collective_compute(
                        "ReduceScatter", ALU.add, replica_groups=GROUPS4,
                        ins=[xpart[cch * 512:(cch + 1) * 512, :].opt()],
                        outs=[xdelta[cch * P:(cch + 1) * P, :].opt()],
                    )
                for st in range(4):
                    xd = sba.tile([P, D], BF16, tag="xd", bufs=2)
                    nc.sync.dma_start(xd[:], xdelta[st * P:(st + 1) * P, :])
                    xdf = sbe.tile([P, D], F32, tag="xdf", bufs=2)
                    nc.vector.tensor_copy(xdf[:], xd[:])
                    nc.vector.tensor_tensor(out=x_tiles[st][:], in0=x_tiles[st][:],
                                            in1=xdf[:], op=ALU.add)
                if f"x{l}a" in tap_outs:
                    for st in range(4):
                        nc.sync.dma_start(tap_outs[f"x{l}a"][st * P:(st + 1) * P, :],
                                          x_tiles[st][:])

                # ---- LN2 + transpose ----
                if flags.ln2_aff:
                    lw2 = sba.tile([P, D], F32, tag="lnw", bufs=2)
                    lb2 = sba.tile([P, D], F32, tag="lnb", bufs=2)
                    nc.sync.dma_start(lw2[:], ln2_in[l, 0])
                    nc.sync.dma_start(lb2[:], ln2_in[l, 1])
                h2T = [sba.tile([P, RPC], BF16, tag=f"h2T{dt}", bufs=1, name=f"h2T{dt}") for dt in range(6)]
                ln_transpose_block((lw2, lb2) if flags.ln2_aff else None, h2T)

                # ---- MLP: h1T = relu(W1^T @ h2 (+b1)); x += h1 @ W2 (+b2) ----
                h1T_all = {}
                if flags.b1_bias:
                    b1t = sba.tile([P, 24], F32, tag="b1t", bufs=2)
                    nc.sync.dma_start(b1t[:], b1_in[l])
                if flags.b2_bias:
                    b2t = sba.tile([1, D], BF16, tag="b2t", bufs=2)
                    nc.sync.dma_start(b2t[:], b2_in[l])
                pxs = []
                for st in range(4):
                    pxa = pst()
                    pxb = pst((P, 256))
                    pxs.append((pxa, pxb))
                h1T_all = {}
                for sg in range(2):
                    for ffh in range(2):
                        if sg == 0:
                            w1_sb = []
                            for kt in range(6):
                                t = sbw.tile([P, FF // 2], BF16, tag=f"w1{kt}",
                                             bufs=1, name=f"w1_{kt}")
                                w1_sb.append(t)
                                nc.sync.dma_start(
                                    t[:], w1_in[l, kt * P:(kt + 1) * P,
                                                ffh * (FF // 2):(ffh + 1) * (FF // 2)])
                            for ft in range(12):
                                fft = ffh * 12 + ft
                                ph = pst()
                                for kt in range(6):
                                    nc.tensor.matmul(
                                        ph[:], w1_sb[kt][:, ft * P:(ft + 1) * P],
                                        h2T[kt][:],
                                        start=(kt == 0), stop=(kt == 5))
                                t = sba.tile([P, RPC], BF16, tag=f"h1T{ft}",
                                             bufs=2, name=f"h1T{fft}")
                                if flags.b1_bias:
                                    nc.scalar.activation(t[:], ph[:], AF.Relu,
                                                         bias=b1t[:, fft:fft + 1])
                                else:
                                    nc.scalar.activation(t[:], ph[:], AF.Relu)
                                h1T_all[fft] = t
                        for ft in range(12):
                            fft = ffh * 12 + ft
                            w2t = sbw.tile([P, D], BF16, tag="w2s", bufs=3)
                            nc.sync.dma_start(w2t[:], w2_in[l, fft * P:(fft + 1) * P, :])
                            for st in (2 * sg, 2 * sg + 1):
                                pxa, pxb = pxs[st]
                                for (px, c0, cn) in ((pxa, 0, 512), (pxb, 512, 256)):
                                    nc.tensor.matmul(
                                        px[:, :cn],
                                        h1T_all[fft][:, st * P:(st + 1) * P],
                                        w2t[:, c0:c0 + cn],
                                        start=(ffh == 0 and ft == 0),
                                        stop=(ffh == 1 and ft == 11
                                              and not flags.b2_bias))
                if flags.b2_bias:
                    for st in range(4):
                        pxa, pxb = pxs[st]
                        for (px, c0, cn) in ((pxa, 0, 512), (pxb, 512, 256)):
                            nc.tensor.matmul(px[:, :cn], onesrow[:], b2t[:, c0:c0 + cn],
                                             start=False, stop=True)
                for st in range(4):
                    pxa, pxb = pxs[st]
                    nc.vector.tensor_tensor(out=x_tiles[st][:, 0:512],
                                            in0=x_tiles[st][:, 0:512],
                                            in1=pxa[:, :], op=ALU.add)
                    nc.vector.tensor_tensor(out=x_tiles[st][:, 512:768],
                                            in0=x_tiles[st][:, 512:768],
                                            in1=pxb[:, :], op=ALU.add)
                if f"x{l}m" in tap_outs:
                    for st in range(4):
                        nc.sync.dma_start(tap_outs[f"x{l}m"][st * P:(st + 1) * P, :],
                                          x_tiles[st][:])

            # ---------------- final LN + AllGather + output projection ----------------
            if flags.lnf_aff:
                lwf = sba.tile([P, D], F32, tag="lnw", bufs=2)
                lbf = sba.tile([P, D], F32, tag="lnb", bufs=2)
                nc.sync.dma_start(lwf[:], lnf_in[0])
                nc.sync.dma_start(lbf[:], lnf_in[1])
            hfTl = [sba.tile([P, RPC], BF16, tag=f"hTl{dt}", bufs=1, name=f"hfTl{dt}") for dt in range(6)]
            ln_transpose_block((lwf, lbf) if flags.lnf_aff else None, hfTl)
            for dt in range(6):
                nc.sync.dma_start(hfT_loc[dt * P:(dt + 1) * P, :], hfTl[dt][:])
            nc.gpsimd.collective_compute(
                "AllGather", ALU.bypass, replica_groups=GROUPS8,
                ins=[hfT_loc[:].opt()], outs=[hfT_gath[:].opt()],
            )

            if flags.out_bias:
                boutt = sba.tile([1, VPC], BF16, tag="boutt", bufs=1)
                nc.sync.dma_start(boutt[:], bout_in[:])
            for sc in range(NC):
                hfc = []
                for kt in range(6):
                    t = sba.tile([P, RPC], BF16, tag=f"hfc{kt}", bufs=1)
                    hfc.append(t)
                    nc.sync.dma_start(
                        t[:], hfT_gath[sc * D + kt * P: sc * D + (kt + 1) * P, :])
                for vc in range(NC):
                    wv_sb = []
                    for kt in range(6):
                        t = sbw.tile([P, VCH], BF16, tag=f"wv{kt}", bufs=2)
                        wv_sb.append(t)
                        nc.sync.dma_start(t[:], wout_in[kt * P:(kt + 1) * P,
                                                        vc * VCH:(vc + 1) * VCH])
                    for st in range(4):
                        pl = pst((P, VCH))
                        for kt in range(6):
                            nc.tensor.matmul(
                                pl[:], hfc[kt][:, st * P:(st + 1) * P], wv_sb[kt][:],
                                start=(kt == 0),
                                stop=(kt == 5 and not flags.out_bias))
                        if flags.out_bias:
                            nc.tensor.matmul(pl[:], onesrow[:],
                                             boutt[:, vc * VCH:(vc + 1) * VCH],
                                             start=False, stop=True)
                        lv = sbe.tile([P, VCH], F32, tag="lv", bufs=2)
                        if st % 2 == 0:
                            nc.scalar.copy(lv[:], pl[:])
                        else:
                            nc.vector.tensor_copy(lv[:], pl[:])
                        nc.sync.dma_start(
                            logits_out[(sc * 4 + st) * P:(sc * 4 + st + 1) * P,
                                       vc * VCH:(vc + 1) * VCH], lv[:])

    nc.compile()
    return nc


_NC_CACHE = {}


def _prep_inputs(inputs, flags):
    tokens = np.asarray(inputs["tokens"]).astype(np.int32)
    emb = np.asarray(inputs["emb"], np.float32)
    Wq, Wk, Wv = (np.asarray(inputs[k], np.float32) for k in ("Wq", "Wk", "Wv"))
    Wo = np.asarray(inputs["Wo"], np.float32)
    bq, bk, bv = (np.asarray(inputs[k], np.float32) for k in ("bq", "bk", "bv"))
    W1, W2 = np.asarray(inputs["W1"], np.float32), np.asarray(inputs["W2"], np.float32)
    b1, b2 = np.asarray(inputs["b1"], np.float32), np.asarray(inputs["b2"], np.float32)
    Wout = np.asarray(inputs["Wout"], np.float32)
    bout = np.asarray(inputs["bout"], np.float32)

    pe_full = _pos_encoding()
    amask = _build_masks()
    ident = np.eye(P, dtype=np.float32).astype(ml_dtypes.bfloat16)
    w1_bf = _bf(W1)
    w2_bf = _bf(W2)

    def col_tiles(vec, n_tiles):
        # [n] -> [128, n_tiles] padded (per-partition bias layout)
        npad = n_tiles * P
        v = np.zeros(npad, np.float32)
        v[:vec.shape[0]] = vec
        return np.ascontiguousarray(v.reshape(n_tiles, P).T)

    in_maps = []
    for c in range(NC):
        b, r = c // 4, c % 4
        # strided ownership: local block j <-> global 128-row block 4j+r
        rows = np.concatenate([np.arange(P) + (4 * j + r) * P for j in range(4)])
        heads = slice(3 * r * DH, 3 * (r + 1) * DH)
        vsh = slice(c * VPC, (c + 1) * VPC)
        wqkv = np.zeros((L, D, D), np.float32)
        wqkv[:, :, 0:192] = Wq[:, :, heads]
        wqkv[:, :, 256:448] = Wk[:, :, heads]
        wqkv[:, :, 512:704] = Wv[:, :, heads]
        m = {
            "emb": emb,
            "tok": np.ascontiguousarray(tokens[b, rows].reshape(4, P, 1)),
            "pe": np.ascontiguousarray(pe_full[rows]),
            "wqkv": _bf(wqkv),
            "wo": _bf(Wo[:, heads, :]),
            "w1": w1_bf,
            "w2": w2_bf,
            "wout": _bf(Wout[:, vsh]),
            "amask": amask,
            "ident": ident,
        }
        if flags.qkv_bias:
            qkvb = np.zeros((L, D), np.float32)
            qkvb[:, 0:192] = bq[:, heads]
            qkvb[:, 256:448] = bk[:, heads]
            qkvb[:, 512:704] = bv[:, heads]
            m["qkvb"] = np.stack([col_tiles(qkvb[l], 6) for l in range(L)])
        if flags.b1_bias:
            m["b1b"] = np.stack([col_tiles(b1[l], 24) for l in range(L)])
        if flags.b2_bias or flags.out_bias:
            m["onesrow"] = np.ones((1, P), ml_dtypes.bfloat16)
        if flags.b2_bias:
            m["b2b"] = _bf(b2.reshape(L, 1, D))
        if flags.out_bias:
            m["boutb"] = _bf(bout[vsh].reshape(1, VPC))
        if flags.ln1_aff:
            m["ln1wb"] = np.ascontiguousarray(np.stack([
                np.broadcast_to(np.asarray(inputs["ln1_w"], np.float32)[:, None, :], (L, P, D)),
                np.broadcast_to(np.asarray(inputs["ln1_b"], np.float32)[:, None, :], (L, P, D))],
                axis=1))
        if flags.ln2_aff:
            m["ln2wb"] = np.ascontiguousarray(np.stack([
                np.broadcast_to(np.asarray(inputs["ln2_w"], np.float32)[:, None, :], (L, P, D)),
                np.broadcast_to(np.asarray(inputs["ln2_b"], np.float32)[:, None, :], (L, P, D))],
                axis=1))
        if flags.lnf_aff:
            m["lnfwb"] = np.ascontiguousarray(np.stack([
                np.broadcast_to(np.asarray(inputs["lnf_w"], np.float32)[None, :], (P, D)),
                np.broadcast_to(np.asarray(inputs["lnf_b"], np.float32)[None, :], (P, D))],
                axis=0))
        in_maps.append(m)
    return in_maps


def make_flags(inputs):
    ln1_w = np.asarray(inputs["ln1_w"]); ln1_b = np.asarray(inputs["ln1_b"])
    ln2_w = np.asarray(inputs["ln2_w"]); ln2_b = np.asarray(inputs["ln2_b"])
    lnf_w = np.asarray(inputs["lnf_w"]); lnf_b = np.asarray(inputs["lnf_b"])
    return Flags(
        qkv_bias=bool(np.any(np.asarray(inputs["bq"])) or np.any(np.asarray(inputs["bk"]))
                      or np.any(np.asarray(inputs["bv"]))),
        b1_bias=bool(np.any(np.asarray(inputs["b1"]))),
        b2_bias=bool(np.any(np.asarray(inputs["b2"]))),
        out_bias=bool(np.any(np.asarray(inputs["bout"]))),
        ln1_aff=not (np.all(ln1_w == 1) and not np.any(ln1_b)),
        ln2_aff=not (np.all(ln2_w == 1) and not np.any(ln2_b)),
        lnf_aff=not (np.all(lnf_w == 1) and not np.any(lnf_b)),
    )


def _row_unperm():
    """kernel logits row kr -> (batch, seq) flat index b*S + s."""
    kr = np.arange(NC * RPC)
    sc = kr // RPC
    b, r = sc // 4, sc % 4
    st = (kr // P) % 4
    i = kr % P
    return b * S + (4 * st + r) * P + i


def run(inputs, taps=()):
    """Run and return (logits, results) — taps add debug outputs."""
    flags = make_flags(inputs)
    key = (flags.key(), tuple(sorted(taps)))
    if key not in _NC_CACHE:
        _NC_CACHE[key] = build_nc(flags, taps)
    nc = _NC_CACHE[key]
    in_maps = _prep_inputs(inputs, flags)
    res = run_bass_kernel_spmd(nc, in_maps, core_ids=list(range(NC)))
    full = np.concatenate([res.results[c]["logits_v2"] for c in range(NC)], axis=-1)
    out = np.empty_like(full)
    out[_row_unperm()] = full
    return out.reshape(B, S, V), res


def kernel(**inputs) -> np.ndarray:
    logits, _ = run(inputs)
    return logits



# revision 12
# speedup vs baseline: 1.3076x; 1.2063x over previous
"""Trainium2 Bass kernel for a 6-layer decoder LM (D=768, H=12, S=2048, B=2, V=32000).

Sharding (8 NeuronCores, one uniform SPMD program):
  core c: batch b=c//4, r=c%4. The core owns the four STRIDED 128-row blocks
  {r, 4+r, 8+r, 12+r} (of 16 blocks per batch) for the residual stream / LN /
  MLP, heads {3r,3r+1,3r+2} for attention, vocab shard [c*4000,(c+1)*4000)
  for the output projection.
  Per layer: LN1 on own rows -> PE-transpose -> AllGather h^T within the batch
  group -> QKV for the core's 3 heads over the full sequence (evicted into
  global column order via a strided AP so attention sees tokens in sequence
  order) -> causal attention in transposed-score layout (keys on partitions,
  exp on ScalarE, softmax denominator via an appended ones-column on V,
  normalization via a rank-1 PE broadcast) -> Wo partial sums in bf16 ->
  4x chunked bf16 ReduceScatter (strided ownership makes each 512-row chunk
  scatter to exactly the owner's local block; chunking keeps each collective
  on the fast <1MB path) -> residual; LN2 -> MLP on own rows -> residual.
  Final: LNf -> AllGather h_f^T over all 8 cores -> vocab-parallel output
  matmul -> [4096, 4000] f32 per core.

All matmuls run in bf16 (weights are cast host-side); residual stream, LN
statistics and PSUM accumulation stay f32.
"""
import math
from contextlib import ExitStack

import numpy as np
import ml_dtypes

import concourse.bacc as bacc
import concourse.bass as bass
import concourse.mybir as mybir
import concourse.tile as tile
from concourse.bass_utils import run_bass_kernel_spmd

V, D, H, L, EPS = 32000, 768, 12, 6, 1e-5
DH = D // H          # 64
FF = 4 * D           # 3072
B, S = 2, 2048
NC = 8
RPC = 512            # rows per core
HPC = 3              # heads per core
VPC = V // NC        # 4000
VCH = 500            # vocab chunk per psum
P = 128
SQRT_D = math.sqrt(D)
F32 = mybir.dt.float32
BF16 = mybir.dt.bfloat16
I32 = mybir.dt.int32
AF = mybir.ActivationFunctionType
ALU = mybir.AluOpType
AX = mybir.AxisListType

GROUPS4 = [[0, 1, 2, 3], [4, 5, 6, 7]]
GROUPS8 = [list(range(NC))]


def _bf(x):
    return np.ascontiguousarray(np.asarray(x, np.float32)).astype(ml_dtypes.bfloat16)


def _build_masks():
    # column block m (of 4): mask[p, m*512 + j] = 1 if j >= m*128 + p else 0
    m = np.zeros((P, 4 * 512), np.float32)
    j = np.arange(512)[None, :]
    p = np.arange(P)[:, None]
    for mm in range(4):
        m[:, mm * 512:(mm + 1) * 512] = (j >= mm * P + p)
    return m.astype(ml_dtypes.bfloat16)


def _pos_encoding():
    pos = np.arange(S, dtype=np.float32)[:, None]
    div = np.exp(np.arange(0, D, 2, dtype=np.float32) * (-math.log(10000.0) / D))
    pe = np.zeros((S, D), dtype=np.float32)
    pe[:, 0::2] = np.sin(pos * div)
    pe[:, 1::2] = np.cos(pos * div)
    return pe


class Flags:
    def __init__(self, qkv_bias, b1_bias, b2_bias, out_bias, ln1_aff, ln2_aff, lnf_aff):
        self.qkv_bias = qkv_bias
        self.b1_bias = b1_bias
        self.b2_bias = b2_bias
        self.out_bias = out_bias
        self.ln1_aff = ln1_aff
        self.ln2_aff = ln2_aff
        self.lnf_aff = lnf_aff

    def key(self):
        return (self.qkv_bias, self.b1_bias, self.b2_bias, self.out_bias,
                self.ln1_aff, self.ln2_aff, self.lnf_aff)


def build_nc(flags: Flags, taps=()):
    nc = bacc.Bacc("TRN2", target_bir_lowering=False, debug=False, num_devices=NC)

    emb_in = nc.dram_tensor("emb", [V, D], F32, kind="ExternalInput")
    tok_in = nc.dram_tensor("tok", [4, P, 1], I32, kind="ExternalInput")
    pe_in = nc.dram_tensor("pe", [RPC, D], F32, kind="ExternalInput")
    wqkv_in = nc.dram_tensor("wqkv", [L, D, D], BF16, kind="ExternalInput")
    wo_in = nc.dram_tensor("wo", [L, HPC * DH, D], BF16, kind="ExternalInput")
    w1_in = nc.dram_tensor("w1", [L, D, FF], BF16, kind="ExternalInput")
    w2_in = nc.dram_tensor("w2", [L, FF, D], BF16, kind="ExternalInput")
    wout_in = nc.dram_tensor("wout", [D, VPC], BF16, kind="ExternalInput")
    amask_in = nc.dram_tensor("amask", [P, 4 * 512], BF16, kind="ExternalInput")
    ident_in = nc.dram_tensor("ident", [P, P], BF16, kind="ExternalInput")
    if flags.qkv_bias:
        qkvb_in = nc.dram_tensor("qkvb", [L, P, 6], F32, kind="ExternalInput")
    if flags.b1_bias:
        b1_in = nc.dram_tensor("b1b", [L, P, 24], F32, kind="ExternalInput")
    if flags.b2_bias or flags.out_bias:
        ones_in = nc.dram_tensor("onesrow", [1, P], BF16, kind="ExternalInput")
    if flags.b2_bias:
        b2_in = nc.dram_tensor("b2b", [L, 1, D], BF16, kind="ExternalInput")
    if flags.out_bias:
        bout_in = nc.dram_tensor("boutb", [1, VPC], BF16, kind="ExternalInput")
    if flags.ln1_aff:
        ln1_in = nc.dram_tensor("ln1wb", [L, 2, P, D], F32, kind="ExternalInput")
    if flags.ln2_aff:
        ln2_in = nc.dram_tensor("ln2wb", [L, 2, P, D], F32, kind="ExternalInput")
    if flags.lnf_aff:
        lnf_in = nc.dram_tensor("lnfwb", [2, P, D], F32, kind="ExternalInput")

    logits_out = nc.dram_tensor("logits_v2", [NC * RPC, VPC], F32, kind="ExternalOutput")
    tap_outs = {}
    for t in taps:
        if t.startswith("x"):
            tap_outs[t] = nc.dram_tensor(f"tap_{t}", [RPC, D], F32, kind="ExternalOutput")
        elif t == "qkvT":
            tap_outs[t] = nc.dram_tensor("tap_qkvT", [D, S], F32, kind="ExternalOutput")
        elif t == "oT":
            tap_outs[t] = nc.dram_tensor("tap_oT", [HPC * DH, S], F32, kind="ExternalOutput")
        elif t == "hT":
            tap_outs[t] = nc.dram_tensor("tap_hT", [D, S], F32, kind="ExternalOutput")

    with tile.TileContext(nc) as tc:
        with ExitStack() as ctx:
            const = ctx.enter_context(tc.tile_pool(name="const", bufs=1))
            dram = ctx.enter_context(tc.tile_pool(name="dram", bufs=1, space="DRAM"))
            xres = ctx.enter_context(tc.tile_pool(name="xres", bufs=1))
            sbw = ctx.enter_context(tc.tile_pool(name="sbw", bufs=1))    # weights
            sba = ctx.enter_context(tc.tile_pool(name="sba", bufs=1))    # activations
            sbe = ctx.enter_context(tc.tile_pool(name="sbe", bufs=2))    # evict/stage
            ps = ctx.enter_context(tc.tile_pool(name="ps", bufs=8, space="PSUM"))

            def pst(shape=(P, 512), dtype=F32):
                return ps.tile(list(shape), dtype, tag="ps", name="pst")

            # ---------------- constants ----------------
            ident = const.tile([P, P], BF16, name="ident")
            nc.sync.dma_start(ident[:], ident_in[:])
            amask = const.tile([P, 4 * 512], BF16, name="amask")
            nc.sync.dma_start(amask[:], amask_in[:])
            ones64 = const.tile([1, DH], BF16, name="ones64")
            nc.vector.memset(ones64[:], 1.0)
            if flags.b2_bias or flags.out_bias:
                onesrow = const.tile([1, P], BF16, name="onesrow")
                nc.sync.dma_start(onesrow[:], ones_in[:])

            def ln_stats(xt, out_bf, wb_tiles=None):
                """LayerNorm of x tile [128, D] f32 -> out_bf [128, D] bf16."""
                mu = sbe.tile([P, 1], F32, tag="mu", bufs=2)
                nc.vector.reduce_sum(mu[:], xt[:], axis=AX.X)
                nc.vector.tensor_scalar_mul(mu[:], mu[:], 1.0 / D)
                xm = sbe.tile([P, D], F32, tag="xm", bufs=1)
                nc.vector.tensor_scalar_sub(xm[:], xt[:], mu[:])
                sq = sbe.tile([P, D], F32, tag="hn", bufs=1, name="sq")
                nc.vector.tensor_tensor(out=sq[:], in0=xm[:], in1=xm[:], op=ALU.mult)
                var = sbe.tile([P, 1], F32, tag="var", bufs=2)
                nc.vector.reduce_sum(var[:], sq[:], axis=AX.X)
                nc.vector.tensor_scalar(var[:], var[:], 1.0 / D, EPS,
                                        op0=ALU.mult, op1=ALU.add)
                rec = sbe.tile([P, 1], F32, tag="rec", bufs=2)
                nc.vector.reciprocal(rec[:], var[:])
                rstd = sbe.tile([P, 1], F32, tag="rstd", bufs=2)
                nc.scalar.sqrt(rstd[:], rec[:])
                if wb_tiles is None:
                    nc.vector.tensor_scalar_mul(out_bf[:], xm[:], rstd[:])
                else:
                    wt, bt = wb_tiles
                    hn = sbe.tile([P, D], F32, tag="hn", bufs=1)
                    nc.vector.tensor_scalar_mul(hn[:], xm[:], rstd[:])
                    nc.vector.tensor_tensor(out=hn[:], in0=hn[:], in1=wt[:], op=ALU.mult)
                    nc.vector.tensor_tensor(out=out_bf[:], in0=hn[:], in1=bt[:], op=ALU.add)

            def transpose_to(dst_sb, src_sb, rows=P):
                """dst_sb [cols, rows] <- src_sb [rows, cols].T  (cols<=128)."""
                cols = src_sb.shape[-1]
                pt = pst((P, P), BF16)
                nc.tensor.transpose(pt[:cols, :rows], src_sb, ident[:rows, :rows])
                nc.scalar.copy(dst_sb, pt[:cols, :rows])

            def ln_transpose_block(wb_tiles, hT_tiles):
                """LN x -> h bf16, then fill hT_tiles[dt][:, st*128...] = h^T."""
                for st in range(4):
                    h_bf = sba.tile([P, D], BF16, tag="h_bf", bufs=2)
                    ln_stats(x_tiles[st], h_bf, wb_tiles)
                    for dt in range(6):
                        transpose_to(hT_tiles[dt][:, st * P:(st + 1) * P],
                                     h_bf[:, dt * P:(dt + 1) * P])

            # ---------------- embedding + positional encoding ----------------
            x_tiles = []
            for st in range(4):
                xt = xres.tile([P, D], F32, name=f"x{st}", tag=f"x{st}", bufs=1)
                x_tiles.append(xt)
                idx = sbe.tile([P, 1], I32, tag="idx", bufs=2)
                nc.sync.dma_start(idx[:], tok_in[st])
                gt = sba.tile([P, D], F32, tag="gt", bufs=1)
                nc.gpsimd.indirect_dma_start(
                    out=gt[:], out_offset=None, in_=emb_in[:],
                    in_offset=bass.IndirectOffsetOnAxis(ap=idx[:, :1], axis=0),
                )
                pt_ = sba.tile([P, D], F32, tag="pt_", bufs=1)
                nc.sync.dma_start(pt_[:], pe_in[st * P:(st + 1) * P, :])
                nc.vector.tensor_scalar_mul(xt[:], gt[:], SQRT_D)
                nc.vector.tensor_tensor(out=xt[:], in0=xt[:], in1=pt_[:], op=ALU.add)

            if "x0" in tap_outs:
                for st in range(4):
                    nc.sync.dma_start(tap_outs["x0"][st * P:(st + 1) * P, :], x_tiles[st][:])

            hT_loc = dram.tile([D, RPC], BF16, name="hT_loc")
            hT_gath = dram.tile([4 * D, RPC], BF16, name="hT_gath")
            xpart = dram.tile([S, D], BF16, name="xpart")
            xdelta = dram.tile([RPC, D], BF16, name="xdelta")
            hfT_loc = dram.tile([D, RPC], BF16, name="hfT_loc")
            # Shared addr_space puts the 8-core AllGather output on the
            # shared-scratchpad fast path (only supported for >4-core AG).
            hfT_gath = dram.tile([NC * D, RPC], BF16, name="hfT_gath",
                                 addr_space="Shared")

            # ---------------- layers ----------------
            for l in range(L):
                # ---- LN1 + transpose + AllGather ----
                if flags.ln1_aff:
                    lw = sba.tile([P, D], F32, tag="lnw", bufs=2)
                    lb = sba.tile([P, D], F32, tag="lnb", bufs=2)
                    nc.sync.dma_start(lw[:], ln1_in[l, 0])
                    nc.sync.dma_start(lb[:], ln1_in[l, 1])
                hTl = [sba.tile([P, RPC], BF16, tag=f"hTl{dt}", bufs=1, name=f"hTl{dt}") for dt in range(6)]
                ln_transpose_block((lw, lb) if flags.ln1_aff else None, hTl)
                for dt in range(6):
                    nc.sync.dma_start(hT_loc[dt * P:(dt + 1) * P, :], hTl[dt][:])
                nc.gpsimd.collective_compute(
                    "AllGather", ALU.bypass, replica_groups=GROUPS4,
                    ins=[hT_loc[:].opt()], outs=[hT_gath[:].opt()],
                )
                # ---- QKV projection (own heads, full sequence) ----
                wq_sb = []
                for kt in range(6):
                    t = sbw.tile([P, D], BF16, tag=f"wq{kt}", bufs=1)
                    wq_sb.append(t)
                    nc.sync.dma_start(t[:], wqkv_in[l, kt * P:(kt + 1) * P, :])
                if flags.qkv_bias:
                    qb = sba.tile([P, 6], F32, tag="qb", bufs=2)
                    nc.sync.dma_start(qb[:], qkvb_in[l])
                # qkvT layout [P, c, s, i]: column (c*4+s)*128+i is global token
                # block 4c+s, so attention sees tokens in global sequence order
                # even though each source core's rows are the strided blocks
                # {s, 4+s, 8+s, 12+s}.
                qkvT = [sba.tile([P, 4, 4, P], BF16, tag=f"qkvT{mt}", bufs=1,
                                 name=f"qkvT{mt}")
                        for mt in range(6)]
                for sc in range(4):
                    hTc = []
                    for kt in range(6):
                        t = sba.tile([P, 512], BF16, tag=f"hTc{kt}", bufs=2)
                        hTc.append(t)
                        nc.sync.dma_start(
                            t[:], hT_gath[sc * D + kt * P: sc * D + (kt + 1) * P, :])
                    for mt in range(6):
                        pq = pst()
                        for kt in range(6):
                            nc.tensor.matmul(
                                pq[:], wq_sb[kt][:, mt * P:(mt + 1) * P], hTc[kt][:],
                                start=(kt == 0), stop=(kt == 5),
                            )
                        dst = qkvT[mt][:, :, sc, :]
                        if flags.qkv_bias:
                            nc.scalar.activation(dst, pq[:], AF.Identity,
                                                 bias=qb[:, mt:mt + 1])
                        else:
                            nc.scalar.copy(dst, pq[:])

                def hrow(base, h):
                    off = base + h * DH
                    return off // P, off % P

                # ---- attention, query-chunk-outer software pipeline ----
                # For each 512-query chunk qc: 3 heads of attention, then that
                # chunk's Wo partials, its bf16 ReduceScatter chunk, residual
                # add and LN2+transpose of the corresponding local block.
                # The RS/LN2 of chunk qc overlap the attention of chunk qc+1.
                oT0 = sba.tile([P, S], BF16, tag="oT0", bufs=1)
                oT1 = sba.tile([DH, S], BF16, tag="oT1", bufs=1)
                wo0 = sbw.tile([P, D], BF16, tag="wo0", bufs=2)
                wo1 = sbw.tile([DH, D], BF16, tag="wo1", bufs=2)
                nc.sync.dma_start(wo0[:], wo_in[l, 0:P, :])
                nc.sync.dma_start(wo1[:], wo_in[l, P:P + DH, :])
                if flags.ln2_aff:
                    lw2 = sba.tile([P, D], F32, tag="lnw", bufs=2)
                    lb2 = sba.tile([P, D], F32, tag="lnb", bufs=2)
                    nc.sync.dma_start(lw2[:], ln2_in[l, 0])
                    nc.sync.dma_start(lb2[:], ln2_in[l, 1])
                h2T = [sba.tile([P, RPC], BF16, tag=f"h2T{dt}", bufs=1,
                                name=f"h2T{dt}") for dt in range(6)]
                vaug_all = {}
                for h in range(HPC):
                    vt_i, vr = hrow(512, h)
                    for kb in range(16):
                        va = sba.tile([P, DH + 1], BF16, tag=f"vaug{h}_{kb}", bufs=1)
                        vaug_all[h, kb] = va
                        pt = pst((P, P), BF16)
                        nc.tensor.transpose(
                            pt[:P, :DH],
                            qkvT[vt_i][vr:vr + DH, kb // 4, kb % 4, :],
                            ident[vr:vr + DH, vr:vr + DH])
                        nc.scalar.copy(va[:, :DH], pt[:P, :DH])
                        nc.vector.memset(va[:, DH:DH + 1], 1.0)
                for qc in range(4):
                    for h in range(HPC):
                        qt_i, qr = hrow(0, h)
                        kt_i, kr = hrow(256, h)
                        po = pst((DH + 1, 512))
                        nkt = 4 * qc + 4
                        for kb in range(nkt):
                            psc = pst()
                            nc.tensor.matmul(
                                psc[:],
                                qkvT[kt_i][kr:kr + DH, kb // 4, kb % 4, :],
                                qkvT[qt_i][qr:qr + DH, qc, :, :],
                                start=True, stop=True,
                            )
                            ex = sba.tile([P, 512], BF16, tag="ex", bufs=3)
                            nc.scalar.activation(ex[:], psc[:], AF.Exp, scale=0.125)
                            m = kb - 4 * qc
                            if m >= 0:
                                nc.vector.tensor_tensor(
                                    out=ex[:], in0=ex[:],
                                    in1=amask[:, m * 512:(m + 1) * 512], op=ALU.mult)
                            nc.tensor.matmul(
                                po[:], vaug_all[h, kb][:], ex[:],
                                start=(kb == 0), stop=(kb == nkt - 1),
                            )
                        rec = sbe.tile([1, 512], F32, tag="recd", bufs=2)
                        nc.vector.reciprocal(rec[:], po[DH:DH + 1, :])
                        recb = sbe.tile([1, 512], BF16, tag="recb", bufs=2)
                        nc.vector.tensor_copy(recb[:], rec[:])
                        pb = pst((DH, 512))
                        nc.tensor.matmul(pb[:], ones64[:], recb[:], start=True, stop=True)
                        bc = sbe.tile([DH, 512], F32, tag="bc", bufs=1)
                        nc.scalar.copy(bc[:], pb[:])
                        dst = oT0[h * DH:(h + 1) * DH, qc * 512:(qc + 1) * 512] \
                            if h < 2 else oT1[:, qc * 512:(qc + 1) * 512]
                        nc.vector.tensor_tensor(out=dst, in0=po[:DH, :], in1=bc[:],
                                                op=ALU.mult)
                    # ---- Wo partials for this chunk's 4 row-tiles (bf16) ----
                    for st4 in range(4):
                        st = 4 * qc + st4
                        sl = slice(st * P, (st + 1) * P)
                        pxa = pst()
                        pxb = pst((P, 256))
                        for (px, c0, cn) in ((pxa, 0, 512), (pxb, 512, 256)):
                            nc.tensor.matmul(px[:, :cn], oT0[:, sl], wo0[:, c0:c0 + cn],
                                             start=True, stop=False)
                            nc.tensor.matmul(px[:, :cn], oT1[:, sl], wo1[:, c0:c0 + cn],
                                             start=False, stop=True)
                        ev = sbe.tile([P, D], BF16, tag="xev", bufs=2)
                        nc.scalar.copy(ev[:, 0:512], pxa[:, :])
                        nc.vector.tensor_copy(ev[:, 512:768], pxb[:, :])
                        nc.sync.dma_start(xpart[sl, :], ev[:])
                    # ---- RS chunk qc: global blocks 4qc..4qc+3 scatter so core
                    # r receives global block 4qc+r == its local block qc ----
                    nc.gpsimd.collective_compute(
                        "ReduceScatter", ALU.add, replica_groups=GROUPS4,
                        ins=[xpart[qc * 512:(qc + 1) * 512, :].opt()],
                        outs=[xdelta[qc * P:(qc + 1) * P, :].opt()],
                    )
                    # ---- residual + LN2 + transpose for local block qc ----
                    xd = sba.tile([P, D], BF16, tag="xd", bufs=2)
                    nc.sync.dma_start(xd[:], xdelta[qc * P:(qc + 1) * P, :])
                    xdf = sbe.tile([P, D], F32, tag="xdf", bufs=2)
                    nc.vector.tensor_copy(xdf[:], xd[:])
                    nc.vector.tensor_tensor(out=x_tiles[qc][:], in0=x_tiles[qc][:],
                                            in1=xdf[:], op=ALU.add)
                    h_bf2 = sba.tile([P, D], BF16, tag="h_bf", bufs=2)
                    ln_stats(x_tiles[qc], h_bf2,
                             (lw2, lb2) if flags.ln2_aff else None)
                    for dt in range(6):
                        transpose_to(h2T[dt][:, qc * P:(qc + 1) * P],
                                     h_bf2[:, dt * P:(dt + 1) * P])
                if f"x{l}a" in tap_outs:
                    for st in range(4):
                        nc.sync.dma_start(tap_outs[f"x{l}a"][st * P:(st + 1) * P, :],
                                          x_tiles[st][:])

                # ---- MLP: h1T = relu(W1^T @ h2 (+b1)); x += h1 @ W2 (+b2) ----
                h1T_all = {}
                if flags.b1_bias:
                    b1t = sba.tile([P, 24], F32, tag="b1t", bufs=2)
                    nc.sync.dma_start(b1t[:], b1_in[l])
                if flags.b2_bias:
                    b2t = sba.tile([1, D], BF16, tag="b2t", bufs=2)
                    nc.sync.dma_start(b2t[:], b2_in[l])
                pxs = []
                for st in range(4):
                    pxa = pst()
                    pxb = pst((P, 256))
                    pxs.append((pxa, pxb))
                h1T_all = {}
                for sg in range(2):
                    for ffh in range(2):
                        if sg == 0:
                            w1_sb = []
                            for kt in range(6):
                                t = sbw.tile([P, FF // 2], BF16, tag=f"w1{kt}",
                                             bufs=1, name=f"w1_{kt}")
                                w1_sb.append(t)
                                nc.sync.dma_start(
                                    t[:], w1_in[l, kt * P:(kt + 1) * P,
                                                ffh * (FF // 2):(ffh + 1) * (FF // 2)])
                            for ft in range(12):
                                fft = ffh * 12 + ft
                                ph = pst()
                                for kt in range(6):
                                    nc.tensor.matmul(
                                        ph[:], w1_sb[kt][:, ft * P:(ft + 1) * P],
                                        h2T[kt][:],
                                        start=(kt == 0), stop=(kt == 5))
                                t = sba.tile([P, RPC], BF16, tag=f"h1T{ft}",
                                             bufs=2, name=f"h1T{fft}")
                                if flags.b1_bias:
                                    nc.scalar.activation(t[:], ph[:], AF.Relu,
                                                         bias=b1t[:, fft:fft + 1])
                                else:
                                    nc.scalar.activation(t[:], ph[:], AF.Relu)
                                h1T_all[fft] = t
                        for ft in range(12):
                            fft = ffh * 12 + ft
                            w2t = sbw.tile([P, D], BF16, tag="w2s", bufs=3)
                            nc.sync.dma_start(w2t[:], w2_in[l, fft * P:(fft + 1) * P, :])
                            for st in (2 * sg, 2 * sg + 1):
                                pxa, pxb = pxs[st]
                                for (px, c0, cn) in ((pxa, 0, 512), (pxb, 512, 256)):
                                    nc.tensor.matmul(
                                        px[:, :cn],
                                        h1T_all[fft][:, st * P:(st + 1) * P],
                                        w2t[:, c0:c0 + cn],
                                        start=(ffh == 0 and ft == 0),
                                        stop=(ffh == 1 and ft == 11
                                              and not flags.b2_bias))
                if flags.b2_bias:
                    for st in range(4):
                        pxa, pxb = pxs[st]
                        for (px, c0, cn) in ((pxa, 0, 512), (pxb, 512, 256)):
                            nc.tensor.matmul(px[:, :cn], onesrow[:], b2t[:, c0:c0 + cn],
                                             start=False, stop=True)
                for st in range(4):
                    pxa, pxb = pxs[st]
                    nc.vector.tensor_tensor(out=x_tiles[st][:, 0:512],
                                            in0=x_tiles[st][:, 0:512],
                                            in1=pxa[:, :], op=ALU.add)
                    nc.vector.tensor_tensor(out=x_tiles[st][:, 512:768],
                                            in0=x_tiles[st][:, 512:768],
                                            in1=pxb[:, :], op=ALU.add)
                if f"x{l}m" in tap_outs:
                    for st in range(4):
                        nc.sync.dma_start(tap_outs[f"x{l}m"][st * P:(st + 1) * P, :],
                                          x_tiles[st][:])

            # ---------------- final LN + AllGather + output projection ----------------
            if flags.lnf_aff:
                lwf = sba.tile([P, D], F32, tag="lnw", bufs=2)
                lbf = sba.tile([P, D], F32, tag="lnb", bufs=2)
                nc.sync.dma_start(lwf[:], lnf_in[0])
                nc.sync.dma_start(lbf[:], lnf_in[1])
            hfTl = [sba.tile([P, RPC], BF16, tag=f"hTl{dt}", bufs=1, name=f"hfTl{dt}") for dt in range(6)]
            ln_transpose_block((lwf, lbf) if flags.lnf_aff else None, hfTl)
            for dt in range(6):
                nc.sync.dma_start(hfT_loc[dt * P:(dt + 1) * P, :], hfTl[dt][:])
            nc.gpsimd.collective_compute(
                "AllGather", ALU.bypass, replica_groups=GROUPS8,
                ins=[hfT_loc[:].opt()], outs=[hfT_gath[:].opt()],
            )

            if flags.out_bias:
                boutt = sba.tile([1, VPC], BF16, tag="boutt", bufs=1)
                nc.sync.dma_start(boutt[:], bout_in[:])
            for sc in range(NC):
                hfc = []
                for kt in range(6):
                    t = sba.tile([P, RPC], BF16, tag=f"hfc{kt}", bufs=1)
                    hfc.append(t)
                    nc.sync.dma_start(
                        t[:], hfT_gath[sc * D + kt * P: sc * D + (kt + 1) * P, :])
                for vc in range(NC):
                    wv_sb = []
                    for kt in range(6):
                        t = sbw.tile([P, VCH], BF16, tag=f"wv{kt}", bufs=2)
                        wv_sb.append(t)
                        nc.sync.dma_start(t[:], wout_in[kt * P:(kt + 1) * P,
                                                        vc * VCH:(vc + 1) * VCH])
                    for st in range(4):
                        pl = pst((P, VCH))
                        for kt in range(6):
                            nc.tensor.matmul(
                                pl[:], hfc[kt][:, st * P:(st + 1) * P], wv_sb[kt][:],
                                start=(kt == 0),
                                stop=(kt == 5 and not flags.out_bias))
                        if flags.out_bias:
                            nc.tensor.matmul(pl[:], onesrow[:],
                                             boutt[:, vc * VCH:(vc + 1) * VCH],
                                             start=False, stop=True)
                        lv = sbe.tile([P, VCH], F32, tag="lv", bufs=2)
                        if st % 2 == 0:
                            nc.scalar.copy(lv[:], pl[:])
                        else:
                            nc.vector.tensor_copy(lv[:], pl[:])
                        nc.sync.dma_start(
                            logits_out[(sc * 4 + st) * P:(sc * 4 + st + 1) * P,
                                       vc * VCH:(vc + 1) * VCH], lv[:])

    nc.compile()
    return nc


_NC_CACHE = {}


def _prep_inputs(inputs, flags):
    tokens = np.asarray(inputs["tokens"]).astype(np.int32)
    emb = np.asarray(inputs["emb"], np.float32)
    Wq, Wk, Wv = (np.asarray(inputs[k], np.float32) for k in ("Wq", "Wk", "Wv"))
    Wo = np.asarray(inputs["Wo"], np.float32)
    bq, bk, bv = (np.asarray(inputs[k], np.float32) for k in ("bq", "bk", "bv"))
    W1, W2 = np.asarray(inputs["W1"], np.float32), np.asarray(inputs["W2"], np.float32)
    b1, b2 = np.asarray(inputs["b1"], np.float32), np.asarray(inputs["b2"], np.float32)
    Wout = np.asarray(inputs["Wout"], np.float32)
    bout = np.asarray(inputs["bout"], np.float32)

    pe_full = _pos_encoding()
    amask = _build_masks()
    ident = np.eye(P, dtype=np.float32).astype(ml_dtypes.bfloat16)
    w1_bf = _bf(W1)
    w2_bf = _bf(W2)

    def col_tiles(vec, n_tiles):
        # [n] -> [128, n_tiles] padded (per-partition bias layout)
        npad = n_tiles * P
        v = np.zeros(npad, np.float32)
        v[:vec.shape[0]] = vec
        return np.ascontiguousarray(v.reshape(n_tiles, P).T)

    in_maps = []
    for c in range(NC):
        b, r = c // 4, c % 4
        # strided ownership: local block j <-> global 128-row block 4j+r
        rows = np.concatenate([np.arange(P) + (4 * j + r) * P for j in range(4)])
        heads = slice(3 * r * DH, 3 * (r + 1) * DH)
        vsh = slice(c * VPC, (c + 1) * VPC)
        wqkv = np.zeros((L, D, D), np.float32)
        wqkv[:, :, 0:192] = Wq[:, :, heads]
        wqkv[:, :, 256:448] = Wk[:, :, heads]
        wqkv[:, :, 512:704] = Wv[:, :, heads]
        m = {
            "emb": emb,
            "tok": np.ascontiguousarray(tokens[b, rows].reshape(4, P, 1)),
            "pe": np.ascontiguousarray(pe_full[rows]),
            "wqkv": _bf(wqkv),
            "wo": _bf(Wo[:, heads, :]),
            "w1": w1_bf,
            "w2": w2_bf,
            "wout": _bf(Wout[:, vsh]),
            "amask": amask,
            "ident": ident,
        }
        if flags.qkv_bias:
            qkvb = np.zeros((L, D), np.float32)
            qkvb[:, 0:192] = bq[:, heads]
            qkvb[:, 256:448] = bk[:, heads]
            qkvb[:, 512:704] = bv[:, heads]
            m["qkvb"] = np.stack([col_tiles(qkvb[l], 6) for l in range(L)])
        if flags.b1_bias:
            m["b1b"] = np.stack([col_tiles(b1[l], 24) for l in range(L)])
        if flags.b2_bias or flags.out_bias:
            m["onesrow"] = np.ones((1, P), ml_dtypes.bfloat16)
        if flags.b2_bias:
            m["b2b"] = _bf(b2.reshape(L, 1, D))
        if flags.out_bias:
            m["boutb"] = _bf(bout[vsh].reshape(1, VPC))
        if flags.ln1_aff:
            m["ln1wb"] = np.ascontiguousarray(np.stack([
                np.broadcast_to(np.asarray(inputs["ln1_w"], np.float32)[:, None, :], (L, P, D)),
                np.broadcast_to(np.asarray(inputs["ln1_b"], np.float32)[:, None, :], (L, P, D))],
                axis=1))
        if flags.ln2_aff:
            m["ln2wb"] = np.ascontiguousarray(np.stack([
                np.broadcast_to(np.asarray(inputs["ln2_w"], np.float32)[:, None, :], (L, P, D)),
                np.broadcast_to(np.asarray(inputs["ln2_b"], np.float32)[:, None, :], (L, P, D))],
                axis=1))
        if flags.lnf_aff:
            m["lnfwb"] = np.ascontiguousarray(np.stack([
                np.broadcast_to(np.asarray(inputs["lnf_w"], np.float32)[None, :], (P, D)),
                np.broadcast_to(np.asarray(inputs["lnf_b"], np.float32)[None, :], (P, D))],
                axis=0))
        in_maps.append(m)
    return in_maps


def make_flags(inputs):
    ln1_w = np.asarray(inputs["ln1_w"]); ln1_b = np.asarray(inputs["ln1_b"])
    ln2_w = np.asarray(inputs["ln2_w"]); ln2_b = np.asarray(inputs["ln2_b"])
    lnf_w = np.asarray(inputs["lnf_w"]); lnf_b = np.asarray(inputs["lnf_b"])
    return Flags(
        qkv_bias=bool(np.any(np.asarray(inputs["bq"])) or np.any(np.asarray(inputs["bk"]))
                      or np.any(np.asarray(inputs["bv"]))),
        b1_bias=bool(np.any(np.asarray(inputs["b1"]))),
        b2_bias=bool(np.any(np.asarray(inputs["b2"]))),
        out_bias=bool(np.any(np.asarray(inputs["bout"]))),
        ln1_aff=not (np.all(ln1_w == 1) and not np.any(ln1_b)),
        ln2_aff=not (np.all(ln2_w == 1) and not np.any(ln2_b)),
        lnf_aff=not (np.all(lnf_w == 1) and not np.any(lnf_b)),
    )


def _row_unperm():
    """kernel logits row kr -> (batch, seq) flat index b*S + s."""
    kr = np.arange(NC * RPC)
    sc = kr // RPC
    b, r = sc // 4, sc % 4
    st = (kr // P) % 4
    i = kr % P
    return b * S + (4 * st + r) * P + i


def run(inputs, taps=()):
    """Run and return (logits, results) — taps add debug outputs."""
    flags = make_flags(inputs)
    key = (flags.key(), tuple(sorted(taps)))
    if key not in _NC_CACHE:
        _NC_CACHE[key] = build_nc(flags, taps)
    nc = _NC_CACHE[key]
    in_maps = _prep_inputs(inputs, flags)
    res = run_bass_kernel_spmd(nc, in_maps, core_ids=list(range(NC)))
    full = np.concatenate([res.results[c]["logits_v2"] for c in range(NC)], axis=-1)
    out = np.empty_like(full)
    out[_row_unperm()] = full
    return out.reshape(B, S, V), res


def kernel(**inputs) -> np.ndarray:
    logits, _ = run(inputs)
    return logits

